# revision 1
# baseline (speedup 1.0000x reference)
"""Trainium2 Bass kernel: 3-layer GCN (AIGEncoder) + global max/sum readout.

8 NeuronCores SPMD. Nodes sharded core = node % 8 (balances per-graph cells
so one compiled schedule serves all cores; per-core structure rides in input
data: index streams + 0/1 reduce matrices). Per layer: bf16 y-table (dinv *
h @ W, node-major) replicated to every core's DRAM (layer 0 built on host
from the 12 distinct embedding rows; layers 1,2 via AllGather); edge
aggregation = chunked dma_gather (edge-major, int16 quarter-local rows) +
TensorEngine 0/1-matrix segment-reduce accumulated in PSUM fp32. ReLU/scales
fold into per-node factors. Readout: gsum via PE graph-mask matmuls, gmax via
DVE max over feature-major h3 (graph-contiguous canonical order); host
combines the 8 partials and rounds.
"""
import sys

sys.path.insert(0, "/opt/trn_rl_repo")

import numpy as np
import ml_dtypes

import concourse.bacc as bacc
import concourse.bass as bass
import concourse.mybir as mybir
from concourse.bass_utils import run_bass_kernel_spmd
from concourse.library_config import mlp

P = 128
N = 100000
NG = 64
D = 128
NC = 8
SHR = None                  # set per-instance: 1 + padded canonical size
CB = 16                     # blocks per chunk
CI = CB * P                 # 2048 idxs per gather
NBUFC = 3                   # chunk pipeline depth


def _host_prep(node_type, ninv, src, dst, batch, emb_type, emb_inv, W0):
    deg = np.bincount(dst, minlength=N) + 1.0
    dinv = (1.0 / np.sqrt(deg)).astype(np.float32)

    cells = np.zeros((NC, NG), dtype=np.int64)
    for c in range(NC):
        cells[c] = np.bincount(batch[np.arange(c, N, NC)], minlength=NG)
    T = cells.max(axis=0)
    cell_start = np.concatenate([[0], np.cumsum(T)])
    ncanon = int(cell_start[-1])
    ntile = -(-ncanon // P)
    ncp = ntile * P

    global SHR
    SHR = ncp + 1
    canon_pos = np.full(N, -1, dtype=np.int64)
    first_of_cell = np.full((NC, NG), -1, dtype=np.int64)
    for c in range(NC):
        nodes_c = np.arange(c, N, NC)
        gs = batch[nodes_c]
        for g in range(NG):
            m = np.flatnonzero(gs == g)
            canon_pos[nodes_c[m]] = cell_start[g] + np.arange(len(m))
            if len(m):
                first_of_cell[c, g] = nodes_c[m[0]]
    trow = (np.arange(N) % NC % 2) * SHR + 1 + canon_pos

    # slot -> (segment rows per quarter); pads duplicate cell's first node
    percore = []
    for c in range(NC):
        slot_node = np.full(ncp, -1, dtype=np.int64)
        nodes_c = np.arange(c, N, NC)
        slot_node[canon_pos[nodes_c]] = nodes_c
        for g in range(NG):
            for j in range(cell_start[g] + cells[c][g], cell_start[g] + T[g]):
                slot_node[j] = first_of_cell[c, g]   # dup (or -1 if empty)
        percore.append(slot_node)

    # per (core, slot) quarter lists
    qs_by_core = []
    for c in range(NC):
        qs = [[[], [], [], []] for _ in range(ncp)]
        sel = np.flatnonzero(dst % NC == c)
        for e in sel:
            s, d = src[e], dst[e]
            qs[canon_pos[d]][(s % NC) // 2].append(int(trow[s]))
        for n in np.arange(c, N, NC):
            qs[canon_pos[n]][(n % NC) // 2].append(int(trow[n]))
        sn = percore[c]
        for j in range(ncp):
            if sn[j] >= 0 and not any(qs[j]):
                # pad slot: duplicate its node's full segment
                nn = sn[j]
                qs[j] = [list(qs[canon_pos[nn]][q]) for q in range(4)]
        qs_by_core.append(qs)

    def need(qs, j):
        return max(len(qs[j][q]) for q in range(4))

    # template blocks per tile
    def blocks_of(qs):
        tiles = []
        for t in range(ntile):
            bl = []
            j = t * P
            while j < (t + 1) * P:
                pos, slots = 0, []
                while j < (t + 1) * P:
                    nd = need(qs, j)
                    if pos + nd > P and slots:
                        break
                    assert pos + nd <= P, "segment too long for one block"
                    slots.append(j)
                    pos += nd
                    j += 1
                bl.append(slots)
            tiles.append(bl)
        return tiles

    all_blocks = [blocks_of(qs_by_core[c]) for c in range(NC)]
    nb_tile = [max(len(all_blocks[c][t]) for c in range(NC)) for t in range(ntile)]
    nblk = sum(nb_tile)
    nblk_pad = -(-nblk // CB) * CB
    nchunk = nblk_pad // CB

    data = []
    for c in range(NC):
        qs = qs_by_core[c]
        idxs = np.zeros((4, nblk_pad * P), dtype=np.int16)
        lhs = np.zeros((nblk_pad, P, P), dtype=ml_dtypes.bfloat16)
        k = 0
        for t in range(ntile):
            bl = all_blocks[c][t]
            for bi in range(nb_tile[t]):
                if bi < len(bl):
                    pos = 0
                    for sj in bl[bi]:
                        nd = need(qs, sj)
                        if nd:
                            lhs[k, pos:pos + nd, sj % P] = 1.0
                            for q in range(4):
                                l = qs[sj][q]
                                idxs[q, k * P + pos:k * P + pos + len(l)] = l
                        pos += nd
                k += 1
        # wrap idx streams into [4, nchunk, P, CI//16]
        wi = np.zeros((4, nchunk, P, CI // 16), dtype=np.int16)
        for q in range(4):
            for ch in range(nchunk):
                part = idxs[q, ch * CI:(ch + 1) * CI]
                w = part.reshape(CI // 16, 16).T
                wi[q, ch] = np.tile(w, (8, 1))
        lhsw = lhs.reshape(nchunk, CB, P, P).transpose(0, 2, 1, 3).reshape(nchunk, P, CB * P).copy()
        data.append(dict(idxs=wi, lhs=lhsw))

    # block k -> tile
    blk_tile = []
    for t in range(ntile):
        blk_tile += [t] * nb_tile[t]
    blk_tile += [ntile - 1] * (nblk_pad - nblk)

    # per-core canonical scalars / masks
    dinvc = np.ones((NC, ncp), dtype=np.float32)
    dinv2 = np.ones((NC, ncp), dtype=np.float32)
    gmask = np.zeros((NC, ncp, NG), dtype=ml_dtypes.bfloat16)
    for c in range(NC):
        nodes_c = np.arange(c, N, NC)
        cp = canon_pos[nodes_c]
        dinvc[c, cp] = dinv[nodes_c]
        dinv2[c, cp] = dinv[nodes_c] ** 2
        gmask[c, cp, batch[nodes_c]] = 1.0

    # layer-0 table: y0[n] = dinv[n] * (emb_type[nt]+emb_inv[ni]) @ W0
    combo = (emb_type[:, None, :] + emb_inv[None, :, :]).reshape(12, D)
    cw = combo @ W0                                  # [12, D]
    y0 = np.zeros((NC, SHR, D), dtype=np.float32)
    cid = node_type * 3 + ninv
    for c in range(NC):
        nodes_c = np.arange(c, N, NC)
        rows = cw[cid[nodes_c]] * dinv[nodes_c][:, None]
        y0[c, 1 + canon_pos[nodes_c] - 0, :] = rows  # canon_pos < SHR-1
    table0 = y0.astype(ml_dtypes.bfloat16)           # [8, SHR, D]

    runs = [(int(cell_start[g]), int(cell_start[g] + T[g])) for g in range(NG)]
    return dict(dinv=dinv, canon_pos=canon_pos, ntile=ntile, ncp=ncp, shr=SHR,
                nb_tile=nb_tile, nblk_pad=nblk_pad, nchunk=nchunk,
                blk_tile=blk_tile, data=data, dinvc=dinvc, dinv2=dinv2,
                gmask=gmask, runs=runs, cells=cells, table0=table0)


def _build(tpl):
    global SHR
    SHR = tpl["shr"]
    ntile, nchunk, nblk_pad = tpl["ntile"], tpl["nchunk"], tpl["nblk_pad"]
    nb_tile, blk_tile, runs = tpl["nb_tile"], tpl["blk_tile"], tpl["runs"]
    assert len(runs) == NG
    dt = mybir.dt
    f32, bf16, i16 = dt.float32, dt.bfloat16, dt.int16

    nc = bacc.Bacc("TRN2", debug=False, num_swdge_queues=2, num_devices=NC)
    w1_d = nc.dram_tensor("w1b", [D, D], bf16, kind="ExternalInput")
    w2_d = nc.dram_tensor("w2b", [D, D], bf16, kind="ExternalInput")
    t0_d = nc.dram_tensor("table0", [NC, SHR, D], bf16, kind="ExternalInput")
    idxs_d = nc.dram_tensor("idxs", [4, nchunk, P, CI // 16], i16, kind="ExternalInput")
    lhs_d = nc.dram_tensor("lhs", [nchunk, P, CB * P], bf16, kind="ExternalInput")
    dinv2_d = nc.dram_tensor("dinv2", [ntile, P], f32, kind="ExternalInput")
    dinvc_d = nc.dram_tensor("dinvc", [ntile, P], f32, kind="ExternalInput")
    gmask_d = nc.dram_tensor("gmask", [ntile, P, NG], bf16, kind="ExternalInput")
    ident_d = nc.dram_tensor("ident", [P, P], bf16, kind="ExternalInput")
    gsum_o = nc.dram_tensor("gsum", [NG, D], f32, kind="ExternalOutput")
    gmax_o = nc.dram_tensor("gmax", [P, NG], f32, kind="ExternalOutput")

    ystage = nc.dram_tensor("ystage", [SHR, D], bf16)
    tables = [t0_d] + [nc.dram_tensor(f"table{l}", [NC, SHR, D], bf16,
                                      addr_space="Shared") for l in (1, 2)]

    tile_k0 = np.cumsum([0] + nb_tile)[:-1]
    tile_k1 = tile_k0 + np.array(nb_tile) - 1
    tile_k1[-1] = nblk_pad - 1

    # ---- compile-time op numbering ----
    mm_idx, ev_idx, vfm_idx, tr_idx, xw_idx, ysc_idx = {}, {}, {}, {}, {}, {}
    pe_n = act_n = dve_n = 0
    for l in range(3):
        for k in range(nblk_pad):
            for q in range(4):
                pe_n += 1
                mm_idx[(l, k, q)] = pe_n
        for t in range(ntile):
            act_n += 1
            ev_idx[(l, t)] = act_n
        for t in range(ntile):
            pe_n += 1
            tr_idx[(l, t)] = pe_n
            act_n += 1
            vfm_idx[(l, t)] = act_n
        for t in range(ntile):
            pe_n += 1
            xw_idx[(l, t)] = pe_n          # l==2: gsum matmuls
            if l < 2:
                dve_n += 1
                ysc_idx[(l, t)] = dve_n

    n_init = 7
    sd_chunk = {}
    SD_Y = {}
    sd = n_init
    for l in range(3):
        for ch in range(nchunk):
            sd += 5
            sd_chunk[(l, ch)] = sd * 16
        if l < 2:
            sd += 1
            SD_Y[l] = sd * 16
    SD_TOTAL = (sd + 2) * 16

    gthr = {}
    cnt = {}
    for l in range(3):
        for ch in range(nchunk):
            for q in range(4):
                key = (q, ch % NBUFC)
                cnt[key] = cnt.get(key, 0) + 16
                gthr[(l, ch, q)] = cnt[key]

    from contextlib import ExitStack
    es = ExitStack()
    block = es.enter_context(nc.Block())
    sb = lambda *a: es.enter_context(nc.sbuf_tensor(*a))
    ps = lambda *a: es.enter_context(nc.psum_tensor(*a))
    sem = lambda n: es.enter_context(nc.semaphore(n))
    bf16_, f32_, i16_ = bf16, f32, i16
    msgs = sb("msgs", [P, 4, NBUFC, CB, D], bf16)
    idx_sb = sb("idx_sb", [P, 4, NBUFC, CI // 16], i16)
    lhs_sb = sb("lhs_sb", [P, NBUFC, CB, P], bf16)
    vnm = sb("vnm", [P, ntile, D], bf16)
    vfm = sb("vfm", [P, ntile, P], bf16)
    ynm = sb("ynm", [P, ntile, D], bf16)
    w_sb = sb("w_sb", [P, 2, D], bf16)
    dinv2_sb = sb("dinv2_sb", [P, ntile], f32)
    dinvc_sb = sb("dinvc_sb", [P, ntile], f32)
    gmask_sb = sb("gmask_sb", [P, ntile, NG], bf16)
    ident_sb = sb("ident_sb", [P, P], bf16)
    gmax_sb = sb("gmax_sb", [P, NG], f32)
    gsum_sb = sb("gsum_sb", [NG, D], f32)
    zrow = sb("zrow", [1, D], bf16)
    ps_red = ps("ps_red", [P, 4, 512], f32)
    ps_aux = ps("ps_aux", [P, 2, 1024], bf16)
    ps_gs = ps("ps_gs", [NG, D], f32)
    sdma = sem("sdma")
    gsem = [[sem(f"g{q}{p}") for p in range(NBUFC)] for q in range(4)]
    pe_s = sem("pe")
    act_s = sem("act")
    dve_s = sem("dve")
    cc_s = sem("cc")
    if True:

        @block.sync
        def _(s):
            _anc = nc.allow_non_contiguous_dma(reason="tiny one-time scale loads")
            _anc.__enter__()
            s.dma_start(w_sb[:, 0, :], w1_d[:]).then_inc(sdma, 16)
            s.dma_start(w_sb[:, 1, :], w2_d[:]).then_inc(sdma, 16)
            s.dma_start(dinv2_sb[:], dinv2_d[:].rearrange("t p -> p t")).then_inc(sdma, 16)
            s.dma_start(dinvc_sb[:], dinvc_d[:].rearrange("t p -> p t")).then_inc(sdma, 16)
            s.dma_start(gmask_sb[:], gmask_d[:].rearrange("t p g -> p t g")).then_inc(sdma, 16)
            s.dma_start(ident_sb[:], ident_d[:]).then_inc(sdma, 16)
            s.wait_ge(dve_s, 1)
            s.dma_start(ystage[0:1, :], zrow[:]).then_inc(sdma, 16)
            _anc.__exit__(None, None, None)
            for l in range(3):
                for ch in range(nchunk):
                    par = ch % NBUFC
                    if ch >= NBUFC:
                        # buffers free when PE consumed chunk ch-NBUFC
                        k_last = (ch - NBUFC) * CB + CB - 1
                        s.wait_ge(pe_s, mm_idx[(l, k_last, 3)])
                    for q in range(4):
                        s.dma_start(idx_sb[:, q, par, :], idxs_d[q, ch]).then_inc(sdma, 16)
                    s.dma_start(lhs_sb[:, par, :, :], lhs_d[ch].rearrange("p (b m) -> p b m", m=P)).then_inc(sdma, 16)
                if l < 2:
                    s.wait_ge(dve_s, 1 + ysc_idx[(l, ntile - 1)])
                    s.dma_start(ystage[1:, :].rearrange("(t p) f -> p t f", p=P), ynm[:]).then_inc(sdma, 16)
            # outputs
            s.wait_ge(dve_s, 1 + dve_n + 1 + NG)          # gmax done (memset + runs)
            s.wait_ge(act_s, act_n + 1)               # gsum evac done
            s.dma_start(gsum_o[:], gsum_sb[:]).then_inc(sdma, 16)
            s.dma_start(gmax_o[:], gmax_sb[:]).then_inc(sdma, 16)
            s.wait_ge(sdma, SD_TOTAL)

        @block.gpsimd
        def _(gp):
            gp.load_library(mlp)
            for l in range(3):
                for ch in range(nchunk):
                    par = ch % NBUFC
                    for q in range(4):
                        if ch >= NBUFC:
                            k_last = (ch - NBUFC) * CB + CB - 1
                            gp.wait_ge(pe_s, mm_idx[(l, k_last, 3)])
                        gp.wait_ge(sdma, sd_chunk[(l, ch)])
                        if l > 0:
                            gp.wait_ge(cc_s, l)
                        gp.dma_gather(
                            msgs[:, q, par, :, :],
                            tables[l][2 * q:2 * q + 2].rearrange("a b c -> (a b) c"),
                            idx_sb[:, q, par, :],
                            CI, CI, D,
                            transpose=False,
                            single_packet=False,
                            queue_num=q % 2,
                        ).then_inc(gsem[q][par], 16)
                if l < 2:
                    gp.wait_ge(sdma, SD_Y[l])
                    gp.collective_compute(
                        "AllGather", mybir.AluOpType.bypass,
                        ins=[ystage[:]], outs=[tables[l + 1][:]],
                        replica_groups=[list(range(NC))],
                    ).then_inc(cc_s, 1)

        @block.tensor
        def _(pe):
            for l in range(3):
                for k in range(nblk_pad):
                    t = blk_tile[k]
                    ch, par = k // CB, (k // CB) % NBUFC
                    for q in range(4):
                        if q == 0 and k % CB == 0:
                            for qq in range(4):
                                pe.wait_ge(gsem[qq][par], gthr[(l, ch, qq)])
                            pe.wait_ge(sdma, sd_chunk[(l, ch)])
                        if k == tile_k0[t] and q == 0 and t >= 4:
                            pe.wait_ge(act_s, ev_idx[(l, t - 4)] if t - 4 >= 0 else 0)
                        pe.matmul(
                            out=ps_red[:, t % 4, :D],
                            lhsT=lhs_sb[:, par, k % CB, :],
                            rhs=msgs[:, q, par, k % CB, :],
                            start=(k == tile_k0[t] and q == 0),
                            stop=(k == tile_k1[t] and q == 3),
                        ).then_inc(pe_s, 1)
                for t in range(ntile):
                    pe.wait_ge(act_s, ev_idx[(l, t)])
                    if t >= 2:
                        pe.wait_ge(act_s, vfm_idx[(l, t - 2)])
                    pe.transpose(out=ps_aux[:, t % 2, :P], in_=vnm[:, t, :],
                                 identity=ident_sb[:]).then_inc(pe_s, 1)
                if l < 2:
                    for t in range(ntile):
                        pe.wait_ge(act_s, vfm_idx[(l, t)])
                        if t >= 4:
                            pe.wait_ge(dve_s, 1 + ysc_idx[(l, t - 4)])
                        pe.matmul(out=ps_red[:, t % 4, :D], lhsT=vfm[:, t, :],
                                  rhs=w_sb[:, l, :], start=True, stop=True,
                                  ).then_inc(pe_s, 1)
                else:
                    for t in range(ntile):
                        pe.wait_ge(act_s, ev_idx[(2, t)])
                        pe.matmul(out=ps_gs[:], lhsT=gmask_sb[:, t, :],
                                  rhs=vnm[:, t, :], start=(t == 0),
                                  stop=(t == ntile - 1)).then_inc(pe_s, 1)

        @block.scalar
        def _(a):
            Act = mybir.ActivationFunctionType
            for l in range(3):
                for t in range(ntile):
                    a.wait_ge(pe_s, mm_idx[(l, int(tile_k1[t]), 3)])
                    if l < 2:
                        a.activation(vnm[:, t, :], ps_red[:, t % 4, :D], Act.Relu).then_inc(act_s, 1)
                    else:
                        a.activation(vnm[:, t, :], ps_red[:, t % 4, :D], Act.Copy,
                                     scale=dinvc_sb[:, t:t + 1]).then_inc(act_s, 1)
                for t in range(ntile):
                    a.wait_ge(pe_s, tr_idx[(l, t)])
                    a.activation(vfm[:, t, :], ps_aux[:, t % 2, :P], Act.Copy).then_inc(act_s, 1)
            a.wait_ge(pe_s, xw_idx[(2, ntile - 1)])
            a.activation(gsum_sb[:], ps_gs[:], Act.Copy).then_inc(act_s, 1)

        @block.vector
        def _(v):
            v.memset(zrow[:], 0.0).then_inc(dve_s, 1)
            for l in range(2):
                for t in range(ntile):
                    v.wait_ge(pe_s, xw_idx[(l, t)])
                    v.tensor_scalar(out=ynm[:, t, :], in0=ps_red[:, t % 4, :D],
                                    scalar1=dinv2_sb[:, t:t + 1], scalar2=None,
                                    op0=mybir.AluOpType.mult).then_inc(dve_s, 1)
            v.memset(gmax_sb[:], -1e30).then_inc(dve_s, 1)
            vf = vfm[:].rearrange("p t f -> p (t f)")
            for g in range(NG):
                a, b = runs[g]
                v.wait_ge(act_s, vfm_idx[(2, (b - 1) // P)])
                v.tensor_reduce(out=gmax_sb[:, g:g + 1], in_=vf[:, a:b],
                                axis=mybir.AxisListType.X,
                                op=mybir.AluOpType.max).then_inc(dve_s, 1)

    es.close()
    nc.compile()
    return nc


def kernel(**inputs):
    node_type = np.asarray(inputs["node_type"]).astype(np.int64)
    ninv = np.asarray(inputs["num_inverted_predecessors"]).astype(np.int64)
    ei = np.asarray(inputs["edge_index"]).astype(np.int64)
    batch = np.asarray(inputs["batch"]).astype(np.int64)
    emb_type = np.asarray(inputs["emb_type"]).astype(np.float32)
    emb_inv = np.asarray(inputs["emb_inv"]).astype(np.float32)
    W0 = np.asarray(inputs["W0"]).astype(np.float32)
    W1 = np.asarray(inputs["W1"]).astype(np.float32)
    W2 = np.asarray(inputs["W2"]).astype(np.float32)

    tpl = _host_prep(node_type, ninv, ei[0], ei[1], batch, emb_type, emb_inv, W0)
    nc = _build(tpl)

    ident = np.eye(P, dtype=ml_dtypes.bfloat16)
    in_maps = []
    for c in range(NC):
        in_maps.append(dict(
            w1b=W1.astype(ml_dtypes.bfloat16),
            w2b=W2.astype(ml_dtypes.bfloat16),
            table0=tpl["table0"],
            idxs=tpl["data"][c]["idxs"],
            lhs=tpl["data"][c]["lhs"],
            dinv2=tpl["dinv2"][c].reshape(tpl["ntile"], P),
            dinvc=tpl["dinvc"][c].reshape(tpl["ntile"], P),
            gmask=tpl["gmask"][c].reshape(tpl["ntile"], P, NG),
            ident=ident,
        ))
    import os
    trace = os.environ.get("BASS_KERNEL_TRACE", "0") == "1"
    if trace:
        sys.path.insert(0, "/root/problem/work")
        try:
            import axon_trace_patch  # noqa
        except Exception:
            trace = False
    res = run_bass_kernel_spmd(nc, in_maps, core_ids=list(range(NC)), trace=trace)
    kernel.last_exec_ns = res.exec_time_ns

    gsum = np.zeros((NG, D), dtype=np.float64)
    gmax = np.full((NG, D), -np.inf)
    for c in range(NC):
        gsum += res.results[c]["gsum"].astype(np.float64)
        gm = res.results[c]["gmax"].astype(np.float64).T   # [NG? no: [P,NG]->T = [NG,P]]
        pres = tpl["cells"][c] > 0
        gmax[pres] = np.maximum(gmax[pres], gm[pres])
    out = np.concatenate([gmax, gsum], axis=1).astype(np.float32)
    return (np.round(out * 1000.0) / 1000.0).astype(np.float32)



# revision 3
# speedup vs baseline: 1.0299x; 1.0299x over previous
"""Trainium2 Bass kernel: 3-layer GCN (AIGEncoder) + global max/sum readout.

8 NeuronCores SPMD. Nodes sharded core = node % 8 (balances per-graph cells
so one compiled schedule serves all cores; per-core structure rides in input
data: index streams + 0/1 reduce matrices). Per layer: bf16 y-table (dinv *
h @ W, node-major) replicated to every core's DRAM (layer 0 built on host
from the 12 distinct embedding rows; layers 1,2 via AllGather); edge
aggregation = chunked dma_gather (edge-major, int16 quarter-local rows) +
TensorEngine 0/1-matrix segment-reduce accumulated in PSUM fp32. ReLU/scales
fold into per-node factors. Readout: gsum via PE graph-mask matmuls, gmax via
DVE max over feature-major h3 (graph-contiguous canonical order); host
combines the 8 partials and rounds.
"""
import sys

sys.path.insert(0, "/opt/trn_rl_repo")

import numpy as np
import ml_dtypes

import concourse.bacc as bacc
import concourse.bass as bass
import concourse.mybir as mybir
from concourse.bass_utils import run_bass_kernel_spmd
from concourse.library_config import mlp

P = 128
N = 100000
NG = 64
D = 128
NC = 8
SHR = None                  # set per-instance: 1 + padded canonical size
CB = 16                     # blocks per chunk
CI = CB * P                 # 2048 idxs per gather
NBUFC = 3                   # chunk pipeline depth


def _host_prep(node_type, ninv, src, dst, batch, emb_type, emb_inv, W0):
    deg = np.bincount(dst, minlength=N) + 1.0
    dinv = (1.0 / np.sqrt(deg)).astype(np.float32)

    cells = np.zeros((NC, NG), dtype=np.int64)
    for c in range(NC):
        cells[c] = np.bincount(batch[np.arange(c, N, NC)], minlength=NG)
    T = cells.max(axis=0)
    cell_start = np.concatenate([[0], np.cumsum(T)])
    ncanon = int(cell_start[-1])
    ntile = -(-ncanon // P)
    ncp = ntile * P

    global SHR
    SHR = ncp + 1
    canon_pos = np.full(N, -1, dtype=np.int64)
    first_of_cell = np.full((NC, NG), -1, dtype=np.int64)
    for c in range(NC):
        nodes_c = np.arange(c, N, NC)
        gs = batch[nodes_c]
        for g in range(NG):
            m = np.flatnonzero(gs == g)
            canon_pos[nodes_c[m]] = cell_start[g] + np.arange(len(m))
            if len(m):
                first_of_cell[c, g] = nodes_c[m[0]]
    trow = (np.arange(N) % NC % 2) * SHR + 1 + canon_pos

    # slot -> (segment rows per quarter); pads duplicate cell's first node
    percore = []
    for c in range(NC):
        slot_node = np.full(ncp, -1, dtype=np.int64)
        nodes_c = np.arange(c, N, NC)
        slot_node[canon_pos[nodes_c]] = nodes_c
        for g in range(NG):
            for j in range(cell_start[g] + cells[c][g], cell_start[g] + T[g]):
                slot_node[j] = first_of_cell[c, g]   # dup (or -1 if empty)
        percore.append(slot_node)

    # per (core, slot) quarter lists
    qs_by_core = []
    for c in range(NC):
        qs = [[[], [], [], []] for _ in range(ncp)]
        sel = np.flatnonzero(dst % NC == c)
        for e in sel:
            s, d = src[e], dst[e]
            qs[canon_pos[d]][(s % NC) // 2].append(int(trow[s]))
        for n in np.arange(c, N, NC):
            qs[canon_pos[n]][(n % NC) // 2].append(int(trow[n]))
        sn = percore[c]
        for j in range(ncp):
            if sn[j] >= 0 and not any(qs[j]):
                # pad slot: duplicate its node's full segment
                nn = sn[j]
                qs[j] = [list(qs[canon_pos[nn]][q]) for q in range(4)]
        qs_by_core.append(qs)

    def need(qs, j):
        return max(len(qs[j][q]) for q in range(4))

    # template blocks per tile
    def blocks_of(qs):
        tiles = []
        for t in range(ntile):
            bl = []
            j = t * P
            while j < (t + 1) * P:
                pos, slots = 0, []
                while j < (t + 1) * P:
                    nd = need(qs, j)
                    if pos + nd > P and slots:
                        break
                    assert pos + nd <= P, "segment too long for one block"
                    slots.append(j)
                    pos += nd
                    j += 1
                bl.append(slots)
            tiles.append(bl)
        return tiles

    all_blocks = [blocks_of(qs_by_core[c]) for c in range(NC)]
    nb_tile = [max(len(all_blocks[c][t]) for c in range(NC)) for t in range(ntile)]
    nblk = sum(nb_tile)
    nblk_pad = -(-nblk // CB) * CB
    nchunk = nblk_pad // CB

    data = []
    for c in range(NC):
        qs = qs_by_core[c]
        idxs = np.zeros((4, nblk_pad * P), dtype=np.int16)
        lhs = np.zeros((nblk_pad, P, P), dtype=ml_dtypes.bfloat16)
        k = 0
        for t in range(ntile):
            bl = all_blocks[c][t]
            for bi in range(nb_tile[t]):
                if bi < len(bl):
                    pos = 0
                    for sj in bl[bi]:
                        nd = need(qs, sj)
                        if nd:
                            lhs[k, pos:pos + nd, sj % P] = 1.0
                            for q in range(4):
                                l = qs[sj][q]
                                idxs[q, k * P + pos:k * P + pos + len(l)] = l
                        pos += nd
                k += 1
        # wrap idx streams into [4, nchunk, P, CI//16]
        wi = np.zeros((4, nchunk, P, CI // 16), dtype=np.int16)
        for q in range(4):
            for ch in range(nchunk):
                part = idxs[q, ch * CI:(ch + 1) * CI]
                w = part.reshape(CI // 16, 16).T
                wi[q, ch] = np.tile(w, (8, 1))
        lhsw = lhs.reshape(nchunk, CB, P, P).transpose(0, 2, 1, 3).reshape(nchunk, P, CB * P).copy()
        data.append(dict(idxs=wi, lhs=lhsw))

    # block k -> tile
    blk_tile = []
    for t in range(ntile):
        blk_tile += [t] * nb_tile[t]
    blk_tile += [ntile - 1] * (nblk_pad - nblk)

    # per-core canonical scalars / masks
    dinvc = np.ones((NC, ncp), dtype=np.float32)
    dinv2 = np.ones((NC, ncp), dtype=np.float32)
    gmask = np.zeros((NC, ncp, NG), dtype=ml_dtypes.bfloat16)
    for c in range(NC):
        nodes_c = np.arange(c, N, NC)
        cp = canon_pos[nodes_c]
        dinvc[c, cp] = dinv[nodes_c]
        dinv2[c, cp] = dinv[nodes_c] ** 2
        gmask[c, cp, batch[nodes_c]] = 1.0

    # layer-0 table: y0[n] = dinv[n] * (emb_type[nt]+emb_inv[ni]) @ W0
    combo = (emb_type[:, None, :] + emb_inv[None, :, :]).reshape(12, D)
    cw = combo @ W0                                  # [12, D]
    y0 = np.zeros((NC, SHR, D), dtype=np.float32)
    cid = node_type * 3 + ninv
    for c in range(NC):
        nodes_c = np.arange(c, N, NC)
        rows = cw[cid[nodes_c]] * dinv[nodes_c][:, None]
        y0[c, 1 + canon_pos[nodes_c] - 0, :] = rows  # canon_pos < SHR-1
    table0 = y0.astype(ml_dtypes.bfloat16)           # [8, SHR, D]

    runs = [(int(cell_start[g]), int(cell_start[g] + T[g])) for g in range(NG)]
    return dict(dinv=dinv, canon_pos=canon_pos, ntile=ntile, ncp=ncp, shr=SHR,
                nb_tile=nb_tile, nblk_pad=nblk_pad, nchunk=nchunk,
                blk_tile=blk_tile, data=data, dinvc=dinvc, dinv2=dinv2,
                gmask=gmask, runs=runs, cells=cells, table0=table0)


def _build(tpl):
    global SHR
    SHR = tpl["shr"]
    ntile, nchunk, nblk_pad = tpl["ntile"], tpl["nchunk"], tpl["nblk_pad"]
    nb_tile, blk_tile, runs = tpl["nb_tile"], tpl["blk_tile"], tpl["runs"]
    assert len(runs) == NG
    dt = mybir.dt
    f32, bf16, i16 = dt.float32, dt.bfloat16, dt.int16

    nc = bacc.Bacc("TRN2", debug=False, num_swdge_queues=4, num_devices=NC)
    w1_d = nc.dram_tensor("w1b", [D, D], bf16, kind="ExternalInput")
    w2_d = nc.dram_tensor("w2b", [D, D], bf16, kind="ExternalInput")
    t0_d = nc.dram_tensor("table0", [NC, SHR, D], bf16, kind="ExternalInput")
    idxs_d = nc.dram_tensor("idxs", [4, nchunk, P, CI // 16], i16, kind="ExternalInput")
    lhs_d = nc.dram_tensor("lhs", [nchunk, P, CB * P], bf16, kind="ExternalInput")
    dinv2_d = nc.dram_tensor("dinv2", [ntile, P], f32, kind="ExternalInput")
    dinvc_d = nc.dram_tensor("dinvc", [ntile, P], f32, kind="ExternalInput")
    gmask_d = nc.dram_tensor("gmask", [ntile, P, NG], bf16, kind="ExternalInput")
    ident_d = nc.dram_tensor("ident", [P, P], bf16, kind="ExternalInput")
    gsum_o = nc.dram_tensor("gsum", [NG, D], f32, kind="ExternalOutput")
    gmax_o = nc.dram_tensor("gmax", [P, NG], f32, kind="ExternalOutput")

    ystage = nc.dram_tensor("ystage", [SHR, D], bf16)
    tables = [t0_d] + [nc.dram_tensor(f"table{l}", [NC, SHR, D], bf16,
                                      addr_space="Shared") for l in (1, 2)]

    tile_k0 = np.cumsum([0] + nb_tile)[:-1]
    tile_k1 = tile_k0 + np.array(nb_tile) - 1
    tile_k1[-1] = nblk_pad - 1

    # ---- compile-time op numbering ----
    mm_idx, ev_idx, vfm_idx, tr_idx, xw_idx, ysc_idx = {}, {}, {}, {}, {}, {}
    pe_n = act_n = dve_n = 0
    for l in range(3):
        for k in range(nblk_pad):
            for q in range(4):
                pe_n += 1
                mm_idx[(l, k, q)] = pe_n
        for t in range(ntile):
            act_n += 1
            ev_idx[(l, t)] = act_n
        for t in range(ntile):
            pe_n += 1
            tr_idx[(l, t)] = pe_n
            act_n += 1
            vfm_idx[(l, t)] = act_n
        for t in range(ntile):
            pe_n += 1
            xw_idx[(l, t)] = pe_n          # l==2: gsum matmuls
            if l < 2:
                dve_n += 1
                ysc_idx[(l, t)] = dve_n

    n_init = 7
    sd_chunk = {}
    SD_Y = {}
    sd = n_init
    for l in range(3):
        for ch in range(nchunk):
            sd += 5
            sd_chunk[(l, ch)] = sd * 16
        if l < 2:
            sd += 1
            SD_Y[l] = sd * 16
    SD_TOTAL = (sd + 2) * 16

    gthr = {}
    cnt = {}
    for l in range(3):
        for ch in range(nchunk):
            for q in range(4):
                key = (q, ch % NBUFC)
                cnt[key] = cnt.get(key, 0) + 16
                gthr[(l, ch, q)] = cnt[key]

    from contextlib import ExitStack
    es = ExitStack()
    block = es.enter_context(nc.Block())
    sb = lambda *a: es.enter_context(nc.sbuf_tensor(*a))
    ps = lambda *a: es.enter_context(nc.psum_tensor(*a))
    sem = lambda n: es.enter_context(nc.semaphore(n))
    bf16_, f32_, i16_ = bf16, f32, i16
    msgs = sb("msgs", [P, 4, NBUFC, CB, D], bf16)
    idx_sb = sb("idx_sb", [P, 4, NBUFC, CI // 16], i16)
    lhs_sb = sb("lhs_sb", [P, NBUFC, CB, P], bf16)
    vnm = sb("vnm", [P, ntile, D], bf16)
    vfm = sb("vfm", [P, ntile, P], bf16)
    ynm = sb("ynm", [P, ntile, D], bf16)
    w_sb = sb("w_sb", [P, 2, D], bf16)
    dinv2_sb = sb("dinv2_sb", [P, ntile], f32)
    dinvc_sb = sb("dinvc_sb", [P, ntile], f32)
    gmask_sb = sb("gmask_sb", [P, ntile, NG], bf16)
    ident_sb = sb("ident_sb", [P, P], bf16)
    gmax_sb = sb("gmax_sb", [P, NG], f32)
    gsum_sb = sb("gsum_sb", [NG, D], f32)
    zrow = sb("zrow", [1, D], bf16)
    ps_red = ps("ps_red", [P, 4, 512], f32)
    ps_aux = ps("ps_aux", [P, 2, 1024], bf16)
    ps_gs = ps("ps_gs", [NG, D], f32)
    sdma = sem("sdma")
    gsem = [[sem(f"g{q}{p}") for p in range(NBUFC)] for q in range(4)]
    pe_s = sem("pe")
    act_s = sem("act")
    dve_s = sem("dve")
    cc_s = sem("cc")
    if True:

        @block.sync
        def _(s):
            _anc = nc.allow_non_contiguous_dma(reason="tiny one-time scale loads")
            _anc.__enter__()
            s.dma_start(w_sb[:, 0, :], w1_d[:]).then_inc(sdma, 16)
            s.dma_start(w_sb[:, 1, :], w2_d[:]).then_inc(sdma, 16)
            s.dma_start(dinv2_sb[:], dinv2_d[:].rearrange("t p -> p t")).then_inc(sdma, 16)
            s.dma_start(dinvc_sb[:], dinvc_d[:].rearrange("t p -> p t")).then_inc(sdma, 16)
            s.dma_start(gmask_sb[:], gmask_d[:].rearrange("t p g -> p t g")).then_inc(sdma, 16)
            s.dma_start(ident_sb[:], ident_d[:]).then_inc(sdma, 16)
            s.wait_ge(dve_s, 1)
            s.dma_start(ystage[0:1, :], zrow[:]).then_inc(sdma, 16)
            _anc.__exit__(None, None, None)
            for l in range(3):
                for ch in range(nchunk):
                    par = ch % NBUFC
                    if ch >= NBUFC:
                        # buffers free when PE consumed chunk ch-NBUFC
                        k_last = (ch - NBUFC) * CB + CB - 1
                        s.wait_ge(pe_s, mm_idx[(l, k_last, 3)])
                    for q in range(4):
                        s.dma_start(idx_sb[:, q, par, :], idxs_d[q, ch]).then_inc(sdma, 16)
                    s.dma_start(lhs_sb[:, par, :, :], lhs_d[ch].rearrange("p (b m) -> p b m", m=P)).then_inc(sdma, 16)
                if l < 2:
                    s.wait_ge(dve_s, 1 + ysc_idx[(l, ntile - 1)])
                    s.dma_start(ystage[1:, :].rearrange("(t p) f -> p t f", p=P), ynm[:]).then_inc(sdma, 16)
            # outputs
            s.wait_ge(dve_s, 1 + dve_n + 1 + NG)          # gmax done (memset + runs)
            s.wait_ge(act_s, act_n + 1)               # gsum evac done
            s.dma_start(gsum_o[:], gsum_sb[:]).then_inc(sdma, 16)
            s.dma_start(gmax_o[:], gmax_sb[:]).then_inc(sdma, 16)
            s.wait_ge(sdma, SD_TOTAL)

        @block.gpsimd
        def _(gp):
            gp.load_library(mlp)
            for l in range(3):
                for ch in range(nchunk):
                    par = ch % NBUFC
                    for q in range(4):
                        if ch >= NBUFC:
                            k_last = (ch - NBUFC) * CB + CB - 1
                            gp.wait_ge(pe_s, mm_idx[(l, k_last, 3)])
                        gp.wait_ge(sdma, sd_chunk[(l, ch)])
                        if l > 0:
                            gp.wait_ge(cc_s, l)
                        gp.dma_gather(
                            msgs[:, q, par, :, :],
                            tables[l][2 * q:2 * q + 2].rearrange("a b c -> (a b) c"),
                            idx_sb[:, q, par, :],
                            CI, CI, D,
                            transpose=False,
                            single_packet=False,
                            queue_num=q,
                        ).then_inc(gsem[q][par], 16)
                if l < 2:
                    gp.wait_ge(sdma, SD_Y[l])
                    gp.collective_compute(
                        "AllGather", mybir.AluOpType.bypass,
                        ins=[ystage[:]], outs=[tables[l + 1][:]],
                        replica_groups=[list(range(NC))],
                    ).then_inc(cc_s, 1)

        @block.tensor
        def _(pe):
            for l in range(3):
                for k in range(nblk_pad):
                    t = blk_tile[k]
                    ch, par = k // CB, (k // CB) % NBUFC
                    for q in range(4):
                        if q == 0 and k % CB == 0:
                            for qq in range(4):
                                pe.wait_ge(gsem[qq][par], gthr[(l, ch, qq)])
                            pe.wait_ge(sdma, sd_chunk[(l, ch)])
                        if k == tile_k0[t] and q == 0 and t >= 4:
                            pe.wait_ge(act_s, ev_idx[(l, t - 4)] if t - 4 >= 0 else 0)
                        pe.matmul(
                            out=ps_red[:, t % 4, :D],
                            lhsT=lhs_sb[:, par, k % CB, :],
                            rhs=msgs[:, q, par, k % CB, :],
                            start=(k == tile_k0[t] and q == 0),
                            stop=(k == tile_k1[t] and q == 3),
                        ).then_inc(pe_s, 1)
                for t in range(ntile):
                    pe.wait_ge(act_s, ev_idx[(l, t)])
                    if t >= 2:
                        pe.wait_ge(act_s, vfm_idx[(l, t - 2)])
                    pe.transpose(out=ps_aux[:, t % 2, :P], in_=vnm[:, t, :],
                                 identity=ident_sb[:]).then_inc(pe_s, 1)
                if l < 2:
                    for t in range(ntile):
                        pe.wait_ge(act_s, vfm_idx[(l, t)])
                        if t >= 4:
                            pe.wait_ge(dve_s, 1 + ysc_idx[(l, t - 4)])
                        pe.matmul(out=ps_red[:, t % 4, :D], lhsT=vfm[:, t, :],
                                  rhs=w_sb[:, l, :], start=True, stop=True,
                                  ).then_inc(pe_s, 1)
                else:
                    for t in range(ntile):
                        pe.wait_ge(act_s, ev_idx[(2, t)])
                        pe.matmul(out=ps_gs[:], lhsT=gmask_sb[:, t, :],
                                  rhs=vnm[:, t, :], start=(t == 0),
                                  stop=(t == ntile - 1)).then_inc(pe_s, 1)

        @block.scalar
        def _(a):
            Act = mybir.ActivationFunctionType
            for l in range(3):
                for t in range(ntile):
                    a.wait_ge(pe_s, mm_idx[(l, int(tile_k1[t]), 3)])
                    if l < 2:
                        a.activation(vnm[:, t, :], ps_red[:, t % 4, :D], Act.Relu).then_inc(act_s, 1)
                    else:
                        a.activation(vnm[:, t, :], ps_red[:, t % 4, :D], Act.Copy,
                                     scale=dinvc_sb[:, t:t + 1]).then_inc(act_s, 1)
                for t in range(ntile):
                    a.wait_ge(pe_s, tr_idx[(l, t)])
                    a.activation(vfm[:, t, :], ps_aux[:, t % 2, :P], Act.Copy).then_inc(act_s, 1)
            a.wait_ge(pe_s, xw_idx[(2, ntile - 1)])
            a.activation(gsum_sb[:], ps_gs[:], Act.Copy).then_inc(act_s, 1)

        @block.vector
        def _(v):
            v.memset(zrow[:], 0.0).then_inc(dve_s, 1)
            for l in range(2):
                for t in range(ntile):
                    v.wait_ge(pe_s, xw_idx[(l, t)])
                    v.tensor_scalar(out=ynm[:, t, :], in0=ps_red[:, t % 4, :D],
                                    scalar1=dinv2_sb[:, t:t + 1], scalar2=None,
                                    op0=mybir.AluOpType.mult).then_inc(dve_s, 1)
            v.memset(gmax_sb[:], -1e30).then_inc(dve_s, 1)
            vf = vfm[:].rearrange("p t f -> p (t f)")
            for g in range(NG):
                a, b = runs[g]
                v.wait_ge(act_s, vfm_idx[(2, (b - 1) // P)])
                v.tensor_reduce(out=gmax_sb[:, g:g + 1], in_=vf[:, a:b],
                                axis=mybir.AxisListType.X,
                                op=mybir.AluOpType.max).then_inc(dve_s, 1)

    es.close()
    nc.compile()
    return nc


def kernel(**inputs):
    node_type = np.asarray(inputs["node_type"]).astype(np.int64)
    ninv = np.asarray(inputs["num_inverted_predecessors"]).astype(np.int64)
    ei = np.asarray(inputs["edge_index"]).astype(np.int64)
    batch = np.asarray(inputs["batch"]).astype(np.int64)
    emb_type = np.asarray(inputs["emb_type"]).astype(np.float32)
    emb_inv = np.asarray(inputs["emb_inv"]).astype(np.float32)
    W0 = np.asarray(inputs["W0"]).astype(np.float32)
    W1 = np.asarray(inputs["W1"]).astype(np.float32)
    W2 = np.asarray(inputs["W2"]).astype(np.float32)

    tpl = _host_prep(node_type, ninv, ei[0], ei[1], batch, emb_type, emb_inv, W0)
    nc = _build(tpl)

    ident = np.eye(P, dtype=ml_dtypes.bfloat16)
    in_maps = []
    for c in range(NC):
        in_maps.append(dict(
            w1b=W1.astype(ml_dtypes.bfloat16),
            w2b=W2.astype(ml_dtypes.bfloat16),
            table0=tpl["table0"],
            idxs=tpl["data"][c]["idxs"],
            lhs=tpl["data"][c]["lhs"],
            dinv2=tpl["dinv2"][c].reshape(tpl["ntile"], P),
            dinvc=tpl["dinvc"][c].reshape(tpl["ntile"], P),
            gmask=tpl["gmask"][c].reshape(tpl["ntile"], P, NG),
            ident=ident,
        ))
    import os
    trace = os.environ.get("BASS_KERNEL_TRACE", "0") == "1"
    if trace:
        sys.path.insert(0, "/root/problem/work")
        try:
            import axon_trace_patch  # noqa
        except Exception:
            trace = False
    res = run_bass_kernel_spmd(nc, in_maps, core_ids=list(range(NC)), trace=trace)
    kernel.last_exec_ns = res.exec_time_ns

    gsum = np.zeros((NG, D), dtype=np.float64)
    gmax = np.full((NG, D), -np.inf)
    for c in range(NC):
        gsum += res.results[c]["gsum"].astype(np.float64)
        gm = res.results[c]["gmax"].astype(np.float64).T   # [NG? no: [P,NG]->T = [NG,P]]
        pres = tpl["cells"][c] > 0
        gmax[pres] = np.maximum(gmax[pres], gm[pres])
    out = np.concatenate([gmax, gsum], axis=1).astype(np.float32)
    return (np.round(out * 1000.0) / 1000.0).astype(np.float32)



# revision 11
# speedup vs baseline: 1.4848x; 1.4416x over previous
"""Trainium2 Bass kernel: 3-layer GCN (AIGEncoder) + global max/sum readout.

8 NeuronCores SPMD, nodes sharded core = node % 8 (canonical per-graph cell
layout shared across cores). Per layer the edge aggregation is a gather of
bf16 y-rows (y = dinv * h @ W, replicated table in DRAM, AllGather between
layers) followed by TensorEngine one-hot segment-reduce matmuls into PSUM.

v2 structure: per-(core, quarter) packed gather streams in dst-canonical
order (per-tile length = max over cores, ~6% padding) on 4 SWDGE queues;
self-loop contribution via PE identity-matmul from SBUF-resident ynm; the
one-hot lhs matrices are generated on-chip (DVE is_equal against an iota
row, overflow share on the Act engine via Square/Relu) from per-partition
slot-id bytes; idx/slot-id data is SBUF-resident, loaded once.
"""
import sys

sys.path.insert(0, "/opt/trn_rl_repo")

import numpy as np
import ml_dtypes

import concourse.bacc as bacc
import concourse.bass as bass
import concourse.mybir as mybir
from concourse.bass_utils import run_bass_kernel_spmd
from concourse.library_config import mlp

P = 128
N = 100000
NG = 64
D = 128
NC = 8
NQ = 4                      # quarters (2 src cores each) = SWDGE queues
CI = 2048                   # rows per gather chunk
CB = CI // P                # windows per chunk
NBUFC = 3                   # msgs chunk ring depth per quarter
NLD = 16                    # DVE-generated lhs ring
NLA = 8                     # ACT-generated lhs ring
ACT_EVERY = 4               # every ACT_EVERY-th agg op's lhs generated on ACT

BF = ml_dtypes.bfloat16


def _host_prep(node_type, ninv, src, dst, batch, emb_type, emb_inv, W0):
    deg = np.bincount(dst, minlength=N) + 1.0
    dinv = (1.0 / np.sqrt(deg)).astype(np.float64)

    cells = np.zeros((NC, NG), dtype=np.int64)
    for c in range(NC):
        cells[c] = np.bincount(batch[np.arange(c, N, NC)], minlength=NG)
    T = cells.max(axis=0)
    cell_start = np.concatenate([[0], np.cumsum(T)])
    ncanon = int(cell_start[-1])
    ntile = -(-ncanon // P)
    ncp = ntile * P

    canon_pos = np.full(N, -1, dtype=np.int64)
    for c in range(NC):
        nodes_c = np.arange(c, N, NC)
        b = batch[nodes_c]
        starts = np.searchsorted(b, np.arange(NG))
        rank = np.arange(len(nodes_c)) - starts[b]
        canon_pos[nodes_c] = cell_start[b] + rank
    trow = (np.arange(N) % NC % 2) * ncp + canon_pos  # row in quarter-pair table

    # ---- per-core edge streams (with pad-slot duplication of cell head) ----
    per_core = []
    for c in range(NC):
        sel = np.flatnonzero(dst % NC == c)
        s_, d_ = src[sel], dst[sel]
        slot = canon_pos[d_]
        qq = (s_ % NC) // 2
        tr = trow[s_]
        # pad slots duplicate the cell-head node's full segment
        ex_slot, ex_q, ex_tr = [], [], []
        order0 = np.lexsort((tr, slot, qq))
        sq_sorted = qq[order0] * ncp + slot[order0]
        for g in range(NG):
            if cells[c][g] == 0 or cells[c][g] == T[g]:
                continue
            f = cell_start[g]
            for q in range(4):
                lo = np.searchsorted(sq_sorted, q * ncp + f)
                hi = np.searchsorted(sq_sorted, q * ncp + f + 1)
                if hi == lo:
                    continue
                rows = tr[order0[lo:hi]]
                for j in range(cell_start[g] + cells[c][g], cell_start[g] + T[g]):
                    ex_slot.append(np.full(hi - lo, j))
                    ex_q.append(np.full(hi - lo, q))
                    ex_tr.append(rows)
        if ex_slot:
            slot = np.concatenate([slot, np.concatenate(ex_slot)])
            qq = np.concatenate([qq, np.concatenate(ex_q)])
            tr = np.concatenate([tr, np.concatenate(ex_tr)])
        per_core.append((slot.astype(np.int64), qq.astype(np.int64),
                         tr.astype(np.int64)))

    seglen = np.zeros((NC, ntile, NQ), dtype=np.int64)
    for c in range(NC):
        slot, qq, _ = per_core[c]
        cnt = np.bincount(slot * NQ + qq, minlength=ncp * NQ).reshape(ncp, NQ)
        seglen[c] = cnt.reshape(ntile, P, NQ).sum(axis=1)
    SEG = seglen.max(axis=0)                       # [ntile, NQ]
    off = np.zeros((NQ, ntile + 1), dtype=np.int64)
    for q in range(NQ):
        off[q, 1:] = np.cumsum(SEG[:, q])
    L = off[:, -1]
    Lpad = -(-L // CI) * CI
    nchunk_q = (Lpad // CI).astype(np.int64)
    nchmax = int(nchunk_q.max())

    # ---- shared op schedule: (t, q, w) agg ops ----
    ops = []                                       # (t, q, w)
    tile_first = np.zeros(ntile, dtype=np.int64)   # index into ops incl ident
    for t in range(ntile):
        tile_first[t] = len(ops)
        for q in range(NQ):
            a, b = int(off[q][t]), int(off[q][t + 1])
            if b > a:
                for w in range(a // P, (b - 1) // P + 1):
                    ops.append((t, q, w))
    n_agg = len(ops)
    # generator assignment (by agg-op index)
    gen_eng = ["act" if (i % ACT_EVERY == ACT_EVERY - 1) else "dve"
               for i in range(n_agg)]
    dve_gi = {}; act_gi = {}
    for i, e in enumerate(gen_eng):
        if e == "dve":
            dve_gi[i] = len(dve_gi)
        else:
            act_gi[i] = len(act_gi)
    n_dve_gen, n_act_gen = len(dve_gi), len(act_gi)

    # ---- per-core payloads ----
    data = []
    for c in range(NC):
        slot, qq, tr = per_core[c]
        order = np.lexsort((tr, slot, qq))
        slot, qq, tr = slot[order], qq[order], tr[order]
        key = qq * ntile + slot // P
        # rank within (q, tile) group
        grp_start = np.searchsorted(key, key, side="left")
        rank = np.arange(len(key)) - grp_start
        pos = off[qq, slot // P] + rank
        slotarr = np.full((NQ, int(Lpad.max())), -1, dtype=np.int32)
        trowarr = np.zeros((NQ, int(Lpad.max())), dtype=np.int64)
        slotarr[qq, pos] = slot
        trowarr[qq, pos] = tr

        # idx wrap: [P, NQ, nchmax, CI//16] int16
        idxh = np.zeros((P, NQ, nchmax, CI // 16), dtype=np.int16)
        for q in range(NQ):
            for ch in range(int(nchunk_q[q])):
                part = trowarr[q, ch * CI:(ch + 1) * CI].astype(np.int16)
                prs = part.reshape(CI // 16, 16)       # [128, 16]
                idxh[:, q, ch, :] = np.tile(prs.T, (8, 1))
        # sid payloads, split per generator engine
        sid_d = np.zeros((P, max(n_dve_gen, 1)), dtype=np.float32)
        sid_a = np.zeros((P, max(n_act_gen, 1)), dtype=np.float32)
        for i, (t, q, w) in enumerate(ops):
            sl = slotarr[q, w * P:(w + 1) * P].astype(np.int64)
            sc = sl - t * P
            sc[(sl < t * P) | (sl >= (t + 1) * P)] = -1
            if gen_eng[i] == "dve":
                sid_d[:, dve_gi[i]] = sc.astype(np.float32)
            else:
                sid_a[:, act_gi[i]] = (-sc).astype(np.float32)
        data.append(dict(idx=idxh, sid_d=sid_d, sid_a=sid_a))

    # ---- per-core canonical scalars / masks / y0 ----
    combo = (emb_type[:, None, :].astype(np.float64)
             + emb_inv[None, :, :].astype(np.float64)).reshape(12, D)
    cw = combo @ W0.astype(np.float64)             # [12, D]
    cid = node_type * 3 + ninv
    dinv2 = np.ones((NC, ncp), dtype=np.float32)
    dinvc = np.ones((NC, ncp), dtype=np.float32)
    gmask = np.zeros((NC, ncp, NG), dtype=BF)
    y0 = np.zeros((NC, ncp, D), dtype=np.float32)
    for c in range(NC):
        nodes_c = np.arange(c, N, NC)
        cp = canon_pos[nodes_c]
        dinv2[c, cp] = (dinv[nodes_c] ** 2).astype(np.float32)
        dinvc[c, cp] = dinv[nodes_c].astype(np.float32)
        gmask[c, cp, batch[nodes_c]] = 1.0
        y0[c, cp, :] = (cw[cid[nodes_c]] * dinv[nodes_c][:, None]).astype(np.float32)
        # pad slots mirror the cell head
        for g in range(NG):
            if cells[c][g] == 0:
                continue
            f = cell_start[g]
            for j in range(cell_start[g] + cells[c][g], cell_start[g] + T[g]):
                dinv2[c, j] = dinv2[c, f]
                dinvc[c, j] = dinvc[c, f]
                y0[c, j, :] = y0[c, f, :]
    table0 = y0.astype(BF)                          # [NC, ncp, D]
    ynm0 = np.zeros((NC, P, ntile, D), dtype=BF)
    for c in range(NC):
        ynm0[c] = table0[c].reshape(ntile, P, D).transpose(1, 0, 2)

    runs = [(int(cell_start[g]), int(cell_start[g + 1])) for g in range(NG)]
    return dict(ntile=ntile, ncp=ncp, ops=ops, gen_eng=gen_eng,
                dve_gi=dve_gi, act_gi=act_gi, n_dve_gen=n_dve_gen,
                n_act_gen=n_act_gen, nchunk_q=[int(x) for x in nchunk_q],
                nchmax=nchmax, data=data, dinv2=dinv2, dinvc=dinvc,
                gmask=gmask, table0=table0, ynm0=ynm0, runs=runs, cells=cells)


def _build(tpl):
    ntile, ncp = tpl["ntile"], tpl["ncp"]
    ops, gen_eng = tpl["ops"], tpl["gen_eng"]
    dve_gi, act_gi = tpl["dve_gi"], tpl["act_gi"]
    n_dve_gen, n_act_gen = tpl["n_dve_gen"], tpl["n_act_gen"]
    nchunk_q, nchmax = tpl["nchunk_q"], tpl["nchmax"]
    runs = tpl["runs"]
    n_agg = len(ops)
    dt = mybir.dt
    f32, bf16, i16 = dt.float32, dt.bfloat16, dt.int16

    # ---------- global numbering ----------
    # PE per layer: per tile [ident + aggs], then ntile tr, then ntile xw/gsum
    n_pe_layer = (n_agg + ntile) + ntile + ntile
    pe_op = {}      # (l, 'ident', t) / (l, 'agg', i) / (l,'tr',t) / (l,'xw',t)
    k = 0
    tile_of_op = [t for (t, q, w) in ops]
    for l in range(3):
        t_seen = -1
        for i, (t, q, w) in enumerate(ops):
            if t != t_seen:
                t_seen = t
                k += 1
                pe_op[(l, 'ident', t)] = k
            k += 1
            pe_op[(l, 'agg', i)] = k
        for t in range(ntile):
            k += 1
            pe_op[(l, 'tr', t)] = k
        for t in range(ntile):
            k += 1
            pe_op[(l, 'xw', t)] = k
    pe_total = k
    # last op index of each tile (for stop flag / evac waits)
    tile_last_op = {}
    for i, (t, q, w) in enumerate(ops):
        tile_last_op[t] = i
    tile_has_agg = set(tile_last_op.keys())

    # ACT per layer: per tile [act-gens, ev], then ntile vfm; l2 end: gsum evac
    act_num = {}
    k = 0
    for l in range(3):
        t_seen = -1
        for i, (t, q, w) in enumerate(ops):
            if t != t_seen:
                # close previous tile
                if t_seen >= 0:
                    k += 1
                    act_num[(l, 'ev', t_seen)] = k
                t_seen = t
            if gen_eng[i] == 'act':
                k += 2  # Square + Relu
                act_num[(l, 'gen', i)] = k
        k += 1
        act_num[(l, 'ev', t_seen)] = k
        for t in range(ntile):
            if t not in tile_has_agg and (l, 'ev', t) not in act_num:
                raise RuntimeError("tile without aggs unsupported")
        for t in range(ntile):
            k += 1
            act_num[(l, 'vfm', t)] = k
    k += 1
    act_num['gsum'] = k
    act_total = k

    # DVE: memset; per layer [gens], then ysc (l<2) / gmax reduces (l==2)
    dve_num = {}
    k = 1  # memset gmax
    for l in range(3):
        for i in range(n_agg):
            if gen_eng[i] == 'dve':
                k += 1
                dve_num[(l, 'gen', i)] = k
        if l < 2:
            for t in range(ntile):
                k += 1
                dve_num[(l, 'ysc', t)] = k
        else:
            for g in range(NG):
                k += 1
                dve_num[(l, 'gmax', g)] = k
    dve_total = k

    # gather numbering: issue order per layer: ch, then q
    gath_n = {}     # (l, q, ch) -> cumulative count on gsem[q]
    cnt_q = [0] * NQ
    for l in range(3):
        for ch in range(nchmax):
            for q in range(NQ):
                if ch < nchunk_q[q]:
                    cnt_q[q] += 1
                    gath_n[(l, q, ch)] = cnt_q[q]

    # msgs buffer recycle: previous consumer pe# per (q, par)
    prev_use = {}
    gath_wait_pe = {}
    # last consuming op (pe#) per (l, q, ch)
    last_pe_of_chunk = {}
    for l in range(3):
        for i, (t, q, w) in enumerate(ops):
            last_pe_of_chunk[(l, q, w // CB)] = pe_op[(l, 'agg', i)]
    for l in range(3):
        for ch in range(nchmax):
            for q in range(NQ):
                if ch >= nchunk_q[q]:
                    continue
                par = ch % NBUFC
                gath_wait_pe[(l, q, ch)] = prev_use.get((q, par), 0)
                prev_use[(q, par)] = last_pe_of_chunk.get(
                    (l, q, ch), prev_use.get((q, par), 0))

    # lhs ring: generator waits pe consumption of slot's previous occupant
    dve_ring_wait = {}
    act_ring_wait = {}
    dve_list = [(l, i) for l in range(3) for i in range(n_agg)
                if gen_eng[i] == 'dve']
    act_list = [(l, i) for l in range(3) for i in range(n_agg)
                if gen_eng[i] == 'act']
    for j, (l, i) in enumerate(dve_list):
        if j >= NLD:
            lp, ip = dve_list[j - NLD]
            dve_ring_wait[(l, i)] = pe_op[(lp, 'agg', ip)]
    for j, (l, i) in enumerate(act_list):
        if j >= NLA:
            lp, ip = act_list[j - NLA]
            act_ring_wait[(l, i)] = pe_op[(lp, 'agg', ip)]
    ring_slot_dve = {(l, i): (dve_gi[i] + l * n_dve_gen) % NLD
                     for (l, i) in dve_list}
    ring_slot_act = {(l, i): (act_gi[i] + l * n_act_gen) % NLA
                     for (l, i) in act_list}

    def tail_tile(b):
        t = ntile - 1
        while t % 4 != b:
            t -= 1
        return t

    # ---------- build ----------
    nc = bacc.Bacc("TRN2", debug=False, num_swdge_queues=4, num_devices=NC)
    w1_d = nc.dram_tensor("w1b", [D, D], bf16, kind="ExternalInput")
    w2_d = nc.dram_tensor("w2b", [D, D], bf16, kind="ExternalInput")
    t0_d = nc.dram_tensor("table0", [NC, ncp, D], bf16, kind="ExternalInput")
    idx_d = nc.dram_tensor("idxh", [P, NQ, nchmax, CI // 16], i16,
                           kind="ExternalInput")
    sidd_d = nc.dram_tensor("sidd", [P, max(n_dve_gen, 1)], f32,
                            kind="ExternalInput")
    sida_d = nc.dram_tensor("sida", [P, max(n_act_gen, 1)], f32,
                            kind="ExternalInput")
    ynm0_d = nc.dram_tensor("ynm0", [P, ntile, D], bf16, kind="ExternalInput")
    dinv2_d = nc.dram_tensor("dinv2", [ntile, P], f32, kind="ExternalInput")
    dinvc_d = nc.dram_tensor("dinvc", [ntile, P], f32, kind="ExternalInput")
    gmask_d = nc.dram_tensor("gmask", [ntile, P, NG], bf16, kind="ExternalInput")
    ident_d = nc.dram_tensor("ident", [P, P], bf16, kind="ExternalInput")
    iota_d = nc.dram_tensor("iota", [P, P], bf16, kind="ExternalInput")
    gsum_o = nc.dram_tensor("gsum", [NG, D], f32, kind="ExternalOutput")
    gmax_o = nc.dram_tensor("gmax", [P, NG], f32, kind="ExternalOutput")
    ystage = nc.dram_tensor("ystage", [ncp, D], bf16)
    tables = [t0_d] + [nc.dram_tensor(f"table{l}", [NC, ncp, D], bf16,
                                      addr_space="Shared") for l in (1, 2)]

    N_INIT = 11                     # initial sync DMA count
    SD_INIT = N_INIT * 16
    SD_YST = {0: (N_INIT + 1) * 16, 1: (N_INIT + 2) * 16}
    SD_TOTAL = (N_INIT + 4) * 16

    from contextlib import ExitStack
    es = ExitStack()
    block = es.enter_context(nc.Block())
    sb = lambda *a: es.enter_context(nc.sbuf_tensor(*a))
    ps = lambda *a: es.enter_context(nc.psum_tensor(*a))
    sem = lambda n: es.enter_context(nc.semaphore(n))

    msgs = sb("msgs", [P, NQ, NBUFC, CB, D], bf16)
    idx_sb = sb("idx_sb", [P, NQ, nchmax, CI // 16], i16)
    sidd_sb = sb("sidd_sb", [P, max(n_dve_gen, 1)], f32)
    sida_sb = sb("sida_sb", [P, max(n_act_gen, 1)], f32)
    lhsd = sb("lhsd", [P, NLD, P], bf16)
    lhsa = sb("lhsa", [P, NLA, P], bf16)
    tmp_sb = sb("tmp_sb", [P, P], bf16)
    vnm = sb("vnm", [P, ntile, D], bf16)
    vfm = sb("vfm", [P, ntile, P], bf16)
    ynm = sb("ynm", [P, ntile, D], bf16)
    w_sb = sb("w_sb", [P, 2, D], bf16)
    dinv2_sb = sb("dinv2_sb", [P, ntile], f32)
    dinvc_sb = sb("dinvc_sb", [P, ntile], f32)
    gmask_sb = sb("gmask_sb", [P, ntile, NG], bf16)
    ident_sb = sb("ident_sb", [P, P], bf16)
    iota_sb = sb("iota_sb", [P, P], bf16)
    gmax_sb = sb("gmax_sb", [P, NG], f32)
    gsum_sb = sb("gsum_sb", [NG, D], f32)
    ps_red = ps("ps_red", [P, 4, 512], f32)
    ps_aux = ps("ps_aux", [P, 2, 1024], bf16)
    ps_gs = ps("ps_gs", [NG, D], f32)
    sdma = sem("sdma")
    gsem = [sem(f"g{q}") for q in range(NQ)]
    pe_s = sem("pe")
    act_s = sem("act")
    dve_s = sem("dve")
    cc_s = sem("cc")

    if True:

        @block.sync
        def _(s):
            _anc = nc.allow_non_contiguous_dma(reason="one-time small loads")
            _anc.__enter__()
            s.dma_start(w_sb[:, 0, :], w1_d[:]).then_inc(sdma, 16)
            s.dma_start(w_sb[:, 1, :], w2_d[:]).then_inc(sdma, 16)
            s.dma_start(dinv2_sb[:], dinv2_d[:].rearrange("t p -> p t")).then_inc(sdma, 16)
            s.dma_start(dinvc_sb[:], dinvc_d[:].rearrange("t p -> p t")).then_inc(sdma, 16)
            s.dma_start(gmask_sb[:], gmask_d[:].rearrange("t p g -> p t g")).then_inc(sdma, 16)
            s.dma_start(ident_sb[:], ident_d[:]).then_inc(sdma, 16)
            s.dma_start(iota_sb[:], iota_d[:]).then_inc(sdma, 16)
            s.dma_start(idx_sb[:], idx_d[:]).then_inc(sdma, 16)
            s.dma_start(sidd_sb[:], sidd_d[:]).then_inc(sdma, 16)
            s.dma_start(sida_sb[:], sida_d[:]).then_inc(sdma, 16)
            s.dma_start(ynm[:], ynm0_d[:]).then_inc(sdma, 16)
            _anc.__exit__(None, None, None)
            for l in range(2):
                s.wait_ge(dve_s, dve_num[(l, 'ysc', ntile - 1)])
                s.dma_start(ystage[:].rearrange("(t p) f -> p t f", p=P),
                            ynm[:]).then_inc(sdma, 16)
            s.wait_ge(dve_s, dve_num[(2, 'gmax', NG - 1)])
            s.wait_ge(act_s, act_num['gsum'])
            s.dma_start(gsum_o[:], gsum_sb[:]).then_inc(sdma, 16)
            s.dma_start(gmax_o[:], gmax_sb[:]).then_inc(sdma, 16)
            s.wait_ge(sdma, SD_TOTAL)

        @block.gpsimd
        def _(gp):
            gp.load_library(mlp)
            gp.wait_ge(sdma, SD_INIT)
            for l in range(3):
                if l > 0:
                    gp.wait_ge(cc_s, l)
                for ch in range(nchmax):
                    for q in range(NQ):
                        if ch >= nchunk_q[q]:
                            continue
                        wpe = gath_wait_pe[(l, q, ch)]
                        if wpe:
                            gp.wait_ge(pe_s, wpe)
                        gp.dma_gather(
                            msgs[:, q, ch % NBUFC, :, :],
                            tables[l][2 * q:2 * q + 2].rearrange(
                                "a b c -> (a b) c"),
                            idx_sb[:, q, ch, :],
                            CI, CI, D,
                            transpose=False,
                            single_packet=False,
                            queue_num=q,
                        ).then_inc(gsem[q], 16)
                if l < 2:
                    gp.wait_ge(sdma, SD_YST[l])
                    gp.collective_compute(
                        "AllGather", mybir.AluOpType.bypass,
                        ins=[ystage[:]], outs=[tables[l + 1][:]],
                        replica_groups=[list(range(NC))],
                    ).then_inc(cc_s, 1)

        @block.tensor
        def _(pe):
            for l in range(3):
                t_seen = -1
                for i, (t, q, w) in enumerate(ops):
                    if t != t_seen:
                        t_seen = t
                        # ident op opens the tile
                        if l == 0:
                            pe.wait_ge(sdma, SD_INIT)
                        else:
                            tb = tail_tile(t % 4) if t < 4 else t
                            pe.wait_ge(dve_s, dve_num[(l - 1, 'ysc', tb)])
                        if t >= 4:
                            pe.wait_ge(act_s, act_num[(l, 'ev', t - 4)])
                        pe.matmul(out=ps_red[:, t % 4, :D], lhsT=ident_sb[:],
                                  rhs=ynm[:, t, :], start=True,
                                  stop=False).then_inc(pe_s, 1)
                    ch = w // CB
                    pe.wait_ge(gsem[q], 16 * gath_n[(l, q, ch)])
                    if gen_eng[i] == 'dve':
                        pe.wait_ge(dve_s, dve_num[(l, 'gen', i)])
                        lhs_ap = lhsd[:, ring_slot_dve[(l, i)], :]
                    else:
                        pe.wait_ge(act_s, act_num[(l, 'gen', i)])
                        lhs_ap = lhsa[:, ring_slot_act[(l, i)], :]
                    pe.matmul(out=ps_red[:, t % 4, :D], lhsT=lhs_ap,
                              rhs=msgs[:, q, ch % NBUFC, w % CB, :],
                              start=False,
                              stop=(i == tile_last_op[t])).then_inc(pe_s, 1)
                for t in range(ntile):
                    pe.wait_ge(act_s, act_num[(l, 'ev', t)])
                    if t >= 2:
                        pe.wait_ge(act_s, act_num[(l, 'vfm', t - 2)])
                    pe.transpose(out=ps_aux[:, t % 2, :P], in_=vnm[:, t, :],
                                 identity=ident_sb[:]).then_inc(pe_s, 1)
                if l < 2:
                    for t in range(ntile):
                        pe.wait_ge(act_s, act_num[(l, 'vfm', t)])
                        if t >= 4:
                            pe.wait_ge(dve_s, dve_num[(l, 'ysc', t - 4)])
                        pe.matmul(out=ps_red[:, t % 4, :D], lhsT=vfm[:, t, :],
                                  rhs=w_sb[:, l, :], start=True,
                                  stop=True).then_inc(pe_s, 1)
                else:
                    for t in range(ntile):
                        pe.wait_ge(act_s, act_num[(2, 'ev', t)])
                        pe.matmul(out=ps_gs[:], lhsT=gmask_sb[:, t, :],
                                  rhs=vnm[:, t, :], start=(t == 0),
                                  stop=(t == ntile - 1)).then_inc(pe_s, 1)

        @block.scalar
        def _(a):
            Act = mybir.ActivationFunctionType
            a.wait_ge(sdma, SD_INIT)
            for l in range(3):
                t_seen = -1
                for i, (t, q, w) in enumerate(ops):
                    if t != t_seen:
                        if t_seen >= 0:
                            a.wait_ge(pe_s, pe_op[(l, 'agg', tile_last_op[t_seen])])
                            if l < 2:
                                a.activation(vnm[:, t_seen, :],
                                             ps_red[:, t_seen % 4, :D],
                                             Act.Relu).then_inc(act_s, 1)
                            else:
                                a.activation(vnm[:, t_seen, :],
                                             ps_red[:, t_seen % 4, :D], Act.Copy,
                                             scale=dinvc_sb[:, t_seen:t_seen + 1]
                                             ).then_inc(act_s, 1)
                        t_seen = t
                    if gen_eng[i] == 'act':
                        wpe = act_ring_wait.get((l, i))
                        if wpe:
                            a.wait_ge(pe_s, wpe)
                        gi = act_gi[i]
                        a.activation(tmp_sb[:], iota_sb[:], Act.Square,
                                     bias=sida_sb[:, gi:gi + 1]).then_inc(act_s, 1)
                        a.activation(lhsa[:, ring_slot_act[(l, i)], :],
                                     tmp_sb[:], Act.Relu, scale=-1.0,
                                     bias=1.0).then_inc(act_s, 1)
                a.wait_ge(pe_s, pe_op[(l, 'agg', tile_last_op[t_seen])])
                if l < 2:
                    a.activation(vnm[:, t_seen, :], ps_red[:, t_seen % 4, :D],
                                 Act.Relu).then_inc(act_s, 1)
                else:
                    a.activation(vnm[:, t_seen, :], ps_red[:, t_seen % 4, :D],
                                 Act.Copy,
                                 scale=dinvc_sb[:, t_seen:t_seen + 1]
                                 ).then_inc(act_s, 1)
                for t in range(ntile):
                    a.wait_ge(pe_s, pe_op[(l, 'tr', t)])
                    a.activation(vfm[:, t, :], ps_aux[:, t % 2, :P],
                                 Act.Copy).then_inc(act_s, 1)
            a.wait_ge(pe_s, pe_op[(2, 'xw', ntile - 1)])
            a.activation(gsum_sb[:], ps_gs[:], Act.Copy).then_inc(act_s, 1)

        @block.vector
        def _(v):
            v.memset(gmax_sb[:], -1e30).then_inc(dve_s, 1)
            v.wait_ge(sdma, SD_INIT)
            for l in range(3):
                for i, (t, q, w) in enumerate(ops):
                    if gen_eng[i] != 'dve':
                        continue
                    wpe = dve_ring_wait.get((l, i))
                    if wpe:
                        v.wait_ge(pe_s, wpe)
                    gi = dve_gi[i]
                    v.tensor_scalar(out=lhsd[:, ring_slot_dve[(l, i)], :],
                                    in0=iota_sb[:],
                                    scalar1=sidd_sb[:, gi:gi + 1],
                                    scalar2=None,
                                    op0=mybir.AluOpType.is_equal
                                    ).then_inc(dve_s, 1)
                if l < 2:
                    for t in range(ntile):
                        v.wait_ge(pe_s, pe_op[(l, 'xw', t)])
                        v.tensor_scalar(out=ynm[:, t, :],
                                        in0=ps_red[:, t % 4, :D],
                                        scalar1=dinv2_sb[:, t:t + 1],
                                        scalar2=None,
                                        op0=mybir.AluOpType.mult
                                        ).then_inc(dve_s, 1)
                else:
                    vf = vfm[:].rearrange("p t f -> p (t f)")
                    for g in range(NG):
                        a0, b0 = runs[g]
                        v.wait_ge(act_s, act_num[(2, 'vfm', (b0 - 1) // P)])
                        v.tensor_reduce(out=gmax_sb[:, g:g + 1],
                                        in_=vf[:, a0:b0],
                                        axis=mybir.AxisListType.X,
                                        op=mybir.AluOpType.max
                                        ).then_inc(dve_s, 1)

    es.close()
    nc.compile()
    return nc


def kernel(**inputs):
    node_type = np.asarray(inputs["node_type"]).astype(np.int64)
    ninv = np.asarray(inputs["num_inverted_predecessors"]).astype(np.int64)
    ei = np.asarray(inputs["edge_index"]).astype(np.int64)
    batch = np.asarray(inputs["batch"]).astype(np.int64)
    emb_type = np.asarray(inputs["emb_type"]).astype(np.float32)
    emb_inv = np.asarray(inputs["emb_inv"]).astype(np.float32)
    W0 = np.asarray(inputs["W0"]).astype(np.float32)
    W1 = np.asarray(inputs["W1"]).astype(np.float32)
    W2 = np.asarray(inputs["W2"]).astype(np.float32)

    tpl = _host_prep(node_type, ninv, ei[0], ei[1], batch, emb_type, emb_inv, W0)
    nc = _build(tpl)

    ident = np.eye(P, dtype=BF)
    iota = np.tile(np.arange(P, dtype=np.float32)[None, :], (P, 1)).astype(BF)
    in_maps = []
    for c in range(NC):
        in_maps.append(dict(
            w1b=W1.astype(BF),
            w2b=W2.astype(BF),
            table0=tpl["table0"],
            idxh=tpl["data"][c]["idx"],
            sidd=tpl["data"][c]["sid_d"],
            sida=tpl["data"][c]["sid_a"],
            ynm0=tpl["ynm0"][c],
            dinv2=tpl["dinv2"][c].reshape(tpl["ntile"], P),
            dinvc=tpl["dinvc"][c].reshape(tpl["ntile"], P),
            gmask=tpl["gmask"][c].reshape(tpl["ntile"], P, NG),
            ident=ident,
            iota=iota,
        ))
    import os
    trace = os.environ.get("BASS_KERNEL_TRACE", "0") == "1"
    if trace:
        sys.path.insert(0, "/root/problem/work")
        try:
            import axon_trace_patch  # noqa
        except Exception:
            trace = False
    res = run_bass_kernel_spmd(nc, in_maps, core_ids=list(range(NC)), trace=trace)
    kernel.last_exec_ns = res.exec_time_ns

    gsum = np.zeros((NG, D), dtype=np.float64)
    gmax = np.full((NG, D), -np.inf)
    for c in range(NC):
        gsum += res.results[c]["gsum"].astype(np.float64)
        gm = res.results[c]["gmax"].astype(np.float64).T   # [NG, P]
        pres = tpl["cells"][c] > 0
        gmax[pres] = np.maximum(gmax[pres], gm[pres])
    out = np.concatenate([gmax, gsum], axis=1).astype(np.float32)
    return (np.round(out * 1000.0) / 1000.0).astype(np.float32)


# revision 14
# speedup vs baseline: 1.5435x; 1.0395x over previous
"""Trainium2 Bass kernel: 3-layer GCN (AIGEncoder) + global max/sum readout.

8 NeuronCores SPMD, nodes sharded core = node % 8 (canonical per-graph cell
layout shared across cores). Per layer the edge aggregation is a gather of
bf16 y-rows (y = dinv * h @ W, replicated table in DRAM, AllGather between
layers) followed by TensorEngine one-hot segment-reduce matmuls into PSUM.

v2 structure: per-(core, quarter) packed gather streams in dst-canonical
order (per-tile length = max over cores, ~6% padding) on 4 SWDGE queues;
self-loop contribution via PE identity-matmul from SBUF-resident ynm; the
one-hot lhs matrices are generated on-chip (DVE is_equal against an iota
row, overflow share on the Act engine via Square/Relu) from per-partition
slot-id bytes; idx/slot-id data is SBUF-resident, loaded once.
"""
import sys

sys.path.insert(0, "/opt/trn_rl_repo")

import numpy as np
import ml_dtypes

import concourse.bacc as bacc
import concourse.bass as bass
import concourse.mybir as mybir
from concourse.bass_utils import run_bass_kernel_spmd
from concourse.library_config import mlp

P = 128
N = 100000
NG = 64
D = 128
NC = 8
NQ = 4                      # quarters (2 src cores each) = SWDGE queues
CI = 2048                   # rows per gather chunk
CB = CI // P                # windows per chunk
NBUFC = 3                   # msgs chunk ring depth per quarter
NLD = 16                    # DVE-generated lhs ring
NLA = 8                     # ACT-generated lhs ring
NLM = 32                    # DMA-loaded lhs ring (matrices)
LCH = 8                     # lhs matrices per DMA chunk
LAG = 6                     # tiles of lag for interleaved tr/xw/ysc
# per-op lhs source pattern (cycled): balance DVE/ACT gen vs DMA load
SRC_PATTERN = ["dve", "act", "dma", "dve", "dma", "act", "dma", "dve", "dma",
               "dve", "dma", "act", "dve", "dma", "dve", "dma", "act", "dma",
               "dve", "dma"]

BF = ml_dtypes.bfloat16


def _host_prep(node_type, ninv, src, dst, batch, emb_type, emb_inv, W0):
    deg = np.bincount(dst, minlength=N) + 1.0
    dinv = (1.0 / np.sqrt(deg)).astype(np.float64)

    cells = np.zeros((NC, NG), dtype=np.int64)
    for c in range(NC):
        cells[c] = np.bincount(batch[np.arange(c, N, NC)], minlength=NG)
    T = cells.max(axis=0)
    cell_start = np.concatenate([[0], np.cumsum(T)])
    ncanon = int(cell_start[-1])
    ntile = -(-ncanon // P)
    ncp = ntile * P

    canon_pos = np.full(N, -1, dtype=np.int64)
    for c in range(NC):
        nodes_c = np.arange(c, N, NC)
        b = batch[nodes_c]
        starts = np.searchsorted(b, np.arange(NG))
        rank = np.arange(len(nodes_c)) - starts[b]
        canon_pos[nodes_c] = cell_start[b] + rank
    trow = (np.arange(N) % NC % 2) * ncp + canon_pos  # row in quarter-pair table

    # ---- per-core edge streams (with pad-slot duplication of cell head) ----
    per_core = []
    for c in range(NC):
        sel = np.flatnonzero(dst % NC == c)
        s_, d_ = src[sel], dst[sel]
        slot = canon_pos[d_]
        qq = (s_ % NC) // 2
        tr = trow[s_]
        # pad slots duplicate the cell-head node's full segment
        ex_slot, ex_q, ex_tr = [], [], []
        order0 = np.lexsort((tr, slot, qq))
        sq_sorted = qq[order0] * ncp + slot[order0]
        for g in range(NG):
            if cells[c][g] == 0 or cells[c][g] == T[g]:
                continue
            f = cell_start[g]
            for q in range(4):
                lo = np.searchsorted(sq_sorted, q * ncp + f)
                hi = np.searchsorted(sq_sorted, q * ncp + f + 1)
                if hi == lo:
                    continue
                rows = tr[order0[lo:hi]]
                for j in range(cell_start[g] + cells[c][g], cell_start[g] + T[g]):
                    ex_slot.append(np.full(hi - lo, j))
                    ex_q.append(np.full(hi - lo, q))
                    ex_tr.append(rows)
        if ex_slot:
            slot = np.concatenate([slot, np.concatenate(ex_slot)])
            qq = np.concatenate([qq, np.concatenate(ex_q)])
            tr = np.concatenate([tr, np.concatenate(ex_tr)])
        per_core.append((slot.astype(np.int64), qq.astype(np.int64),
                         tr.astype(np.int64)))

    seglen = np.zeros((NC, ntile, NQ), dtype=np.int64)
    for c in range(NC):
        slot, qq, _ = per_core[c]
        cnt = np.bincount(slot * NQ + qq, minlength=ncp * NQ).reshape(ncp, NQ)
        seglen[c] = cnt.reshape(ntile, P, NQ).sum(axis=1)
    SEG = seglen.max(axis=0)                       # [ntile, NQ]
    off = np.zeros((NQ, ntile + 1), dtype=np.int64)
    for q in range(NQ):
        off[q, 1:] = np.cumsum(SEG[:, q])
    L = off[:, -1]
    Lpad = -(-L // CI) * CI
    nchunk_q = (Lpad // CI).astype(np.int64)
    nchmax = int(nchunk_q.max())

    # ---- shared op schedule: (t, q, w) agg ops ----
    ops = []                                       # (t, q, w)
    tile_first = np.zeros(ntile, dtype=np.int64)   # index into ops incl ident
    for t in range(ntile):
        tile_first[t] = len(ops)
        for q in range(NQ):
            a, b = int(off[q][t]), int(off[q][t + 1])
            if b > a:
                for w in range(a // P, (b - 1) // P + 1):
                    ops.append((t, q, w))
    n_agg = len(ops)
    # generator assignment (by agg-op index): dve / act / dma pattern
    gen_eng = [SRC_PATTERN[i % len(SRC_PATTERN)] for i in range(n_agg)]
    dve_gi = {}; act_gi = {}; dma_gi = {}
    for i, e in enumerate(gen_eng):
        if e == "dve":
            dve_gi[i] = len(dve_gi)
        elif e == "act":
            act_gi[i] = len(act_gi)
        else:
            dma_gi[i] = len(dma_gi)
    n_dve_gen, n_act_gen, n_dma = len(dve_gi), len(act_gi), len(dma_gi)

    # ---- per-core payloads ----
    data = []
    for c in range(NC):
        slot, qq, tr = per_core[c]
        order = np.lexsort((tr, slot, qq))
        slot, qq, tr = slot[order], qq[order], tr[order]
        key = qq * ntile + slot // P
        # rank within (q, tile) group
        grp_start = np.searchsorted(key, key, side="left")
        rank = np.arange(len(key)) - grp_start
        pos = off[qq, slot // P] + rank
        slotarr = np.full((NQ, int(Lpad.max())), -1, dtype=np.int32)
        trowarr = np.zeros((NQ, int(Lpad.max())), dtype=np.int64)
        slotarr[qq, pos] = slot
        trowarr[qq, pos] = tr

        # idx wrap: [P, NQ, nchmax, CI//16] int16
        idxh = np.zeros((P, NQ, nchmax, CI // 16), dtype=np.int16)
        for q in range(NQ):
            for ch in range(int(nchunk_q[q])):
                part = trowarr[q, ch * CI:(ch + 1) * CI].astype(np.int16)
                prs = part.reshape(CI // 16, 16)       # [128, 16]
                idxh[:, q, ch, :] = np.tile(prs.T, (8, 1))
        # sid payloads, split per generator engine; dma ops get full matrices
        sid_d = np.zeros((P, max(n_dve_gen, 1)), dtype=np.float32)
        sid_a = np.zeros((P, max(n_act_gen, 1)), dtype=np.float32)
        n_dma_ch = -(-max(n_dma, 1) // LCH)
        lhsm = np.zeros((n_dma_ch, P, LCH, P), dtype=BF)
        for i, (t, q, w) in enumerate(ops):
            sl = slotarr[q, w * P:(w + 1) * P].astype(np.int64)
            sc = sl - t * P
            sc[(sl < t * P) | (sl >= (t + 1) * P)] = -1
            if gen_eng[i] == "dve":
                sid_d[:, dve_gi[i]] = sc.astype(np.float32)
            elif gen_eng[i] == "act":
                sid_a[:, act_gi[i]] = (-sc).astype(np.float32)
            else:
                j = dma_gi[i]
                oh = np.zeros((P, P), dtype=BF)
                v = sc >= 0
                oh[np.arange(P)[v], sc[v].astype(np.int64)] = 1.0
                lhsm[j // LCH, :, j % LCH, :] = oh
        data.append(dict(idx=idxh, sid_d=sid_d, sid_a=sid_a, lhsm=lhsm))

    # ---- per-core canonical scalars / masks / y0 ----
    combo = (emb_type[:, None, :].astype(np.float64)
             + emb_inv[None, :, :].astype(np.float64)).reshape(12, D)
    cw = combo @ W0.astype(np.float64)             # [12, D]
    cid = node_type * 3 + ninv
    dinv2 = np.ones((NC, ncp), dtype=np.float32)
    dinvc = np.ones((NC, ncp), dtype=np.float32)
    gmask = np.zeros((NC, ncp, NG), dtype=BF)
    y0 = np.zeros((NC, ncp, D), dtype=np.float32)
    for c in range(NC):
        nodes_c = np.arange(c, N, NC)
        cp = canon_pos[nodes_c]
        dinv2[c, cp] = (dinv[nodes_c] ** 2).astype(np.float32)
        dinvc[c, cp] = dinv[nodes_c].astype(np.float32)
        gmask[c, cp, batch[nodes_c]] = 1.0
        y0[c, cp, :] = (cw[cid[nodes_c]] * dinv[nodes_c][:, None]).astype(np.float32)
        # pad slots mirror the cell head
        for g in range(NG):
            if cells[c][g] == 0:
                continue
            f = cell_start[g]
            for j in range(cell_start[g] + cells[c][g], cell_start[g] + T[g]):
                dinv2[c, j] = dinv2[c, f]
                dinvc[c, j] = dinvc[c, f]
                y0[c, j, :] = y0[c, f, :]
    table0 = y0.astype(BF)                          # [NC, ncp, D]
    ynm0 = np.zeros((NC, P, ntile, D), dtype=BF)
    for c in range(NC):
        ynm0[c] = table0[c].reshape(ntile, P, D).transpose(1, 0, 2)

    runs = [(int(cell_start[g]), int(cell_start[g + 1])) for g in range(NG)]
    return dict(ntile=ntile, ncp=ncp, ops=ops, gen_eng=gen_eng,
                dve_gi=dve_gi, act_gi=act_gi, dma_gi=dma_gi,
                n_dve_gen=n_dve_gen,
                n_act_gen=n_act_gen, n_dma=n_dma,
                nchunk_q=[int(x) for x in nchunk_q],
                nchmax=nchmax, data=data, dinv2=dinv2, dinvc=dinvc,
                gmask=gmask, table0=table0, ynm0=ynm0, runs=runs, cells=cells)


def _build(tpl):
    ntile, ncp = tpl["ntile"], tpl["ncp"]
    ops, gen_eng = tpl["ops"], tpl["gen_eng"]
    dve_gi, act_gi, dma_gi = tpl["dve_gi"], tpl["act_gi"], tpl["dma_gi"]
    n_dve_gen, n_act_gen, n_dma = tpl["n_dve_gen"], tpl["n_act_gen"], tpl["n_dma"]
    nchunk_q, nchmax = tpl["nchunk_q"], tpl["nchmax"]
    runs = tpl["runs"]
    n_agg = len(ops)
    n_dma_ch = -(-max(n_dma, 1) // LCH)
    dt = mybir.dt
    f32, bf16, i16 = dt.float32, dt.bfloat16, dt.int16

    tile_last_op = {}
    tile_ops = {}
    for i, (t, q, w) in enumerate(ops):
        tile_last_op[t] = i
        tile_ops.setdefault(t, []).append(i)
    for t in range(ntile):
        assert t in tile_ops, "tile without aggs unsupported"

    def tail_tile(b):
        t = ntile - 1
        while t % 4 != b:
            t -= 1
        return t

    # ---------- instruction sequences (shared by numbering and emission) ----
    def pe_seq(l):
        out = []
        for t in range(ntile):
            out.append(('ident', t))
            for i in tile_ops[t]:
                out.append(('agg', i))
            if t >= LAG:
                out.append(('tr', t - LAG))
                out.append(('xw', t - LAG))
        for t in range(ntile - LAG, ntile):
            out.append(('tr', t))
            out.append(('xw', t))
        return out

    def act_seq(l):
        out = []
        for t in range(ntile):
            for i in tile_ops[t]:
                if gen_eng[i] == 'act':
                    out.append(('gsq', i))
                    out.append(('grl', i))
            out.append(('ev', t))
            if t >= LAG:
                out.append(('vfm', t - LAG))
        for t in range(ntile - LAG, ntile):
            out.append(('vfm', t))
        return out

    def dve_seq(l):
        out = []
        for t in range(ntile):
            for i in tile_ops[t]:
                if gen_eng[i] == 'dve':
                    out.append(('gen', i))
            if l < 2 and t >= LAG:
                out.append(('ysc', t - LAG))
        if l < 2:
            for t in range(ntile - LAG, ntile):
                out.append(('ysc', t))
        else:
            for g in range(NG):
                out.append(('gmax', g))
        return out

    # ---------- numbering ----------
    pe_num, act_num, dve_num = {}, {}, {}
    k = 0
    for l in range(3):
        for rec in pe_seq(l):
            k += 1
            pe_num[(l,) + rec] = k
    k = 0
    for l in range(3):
        for rec in act_seq(l):
            k += 1
            act_num[(l,) + rec] = k
    k += 1
    act_num['gsum'] = k
    k = 1  # memset
    for l in range(3):
        for rec in dve_seq(l):
            k += 1
            dve_num[(l,) + rec] = k

    # gather numbering (cumulative per quarter sem)
    gath_n = {}
    cnt_q = [0] * NQ
    for l in range(3):
        for ch in range(nchmax):
            for q in range(NQ):
                if ch < nchunk_q[q]:
                    cnt_q[q] += 1
                    gath_n[(l, q, ch)] = cnt_q[q]

    # msgs buffer recycle
    last_pe_of_chunk = {}
    for l in range(3):
        for i, (t, q, w) in enumerate(ops):
            last_pe_of_chunk[(l, q, w // CB)] = pe_num[(l, 'agg', i)]
    gath_wait_pe = {}
    prev_use = {}
    for l in range(3):
        for ch in range(nchmax):
            for q in range(NQ):
                if ch >= nchunk_q[q]:
                    continue
                par = ch % NBUFC
                gath_wait_pe[(l, q, ch)] = prev_use.get((q, par), 0)
                prev_use[(q, par)] = last_pe_of_chunk.get(
                    (l, q, ch), prev_use.get((q, par), 0))

    # generator/DMA lhs rings: slot + writer-wait (pe# of previous occupant)
    def ring_plan(idx_map, n_per_layer, depth):
        slot_of, wait_of = {}, {}
        prev = {}
        for l in range(3):
            for i in sorted(idx_map, key=idx_map.get):
                s = idx_map[i] % depth
                slot_of[(l, i)] = s
                if (ps := prev.get(s)) is not None:
                    wait_of[(l, i)] = pe_num[(ps[0], 'agg', ps[1])]
                prev[s] = (l, i)
        return slot_of, wait_of

    slot_dve, wait_dve = ring_plan(dve_gi, n_dve_gen, NLD)
    slot_act, wait_act = ring_plan(act_gi, n_act_gen, NLA)
    slot_dma, _ = ring_plan(dma_gi, n_dma, NLM)
    # lhs DMA chunks: (l, j) -> wait pe# for ring reuse
    inv_dma = {v: k2 for k2, v in dma_gi.items()}
    chunk_wait = {}
    prev_chunk = {}
    for l in range(3):
        for j in range(n_dma_ch):
            cslot = (j * LCH) % NLM
            if (pc := prev_chunk.get(cslot)) is not None:
                lp, jp = pc
                last = min(jp * LCH + LCH, n_dma) - 1
                chunk_wait[(l, j)] = pe_num[(lp, 'agg', inv_dma[last])]
            prev_chunk[cslot] = (l, j)

    # sync sdma numbering
    N_INIT = 12
    sd = N_INIT
    sd_lhs = {}
    SD_YST = {}
    for l in range(3):
        for j in range(n_dma_ch):
            sd += 1
            sd_lhs[(l, j)] = sd * 16
        if l < 2:
            sd += 1
            SD_YST[l] = sd * 16
    SD_TOTAL = (sd + 2) * 16
    SD_INIT = N_INIT * 16

    # ---------- build ----------
    nc = bacc.Bacc("TRN2", debug=False, num_swdge_queues=4, num_devices=NC)
    w1_d = nc.dram_tensor("w1b", [D, D], bf16, kind="ExternalInput")
    w2_d = nc.dram_tensor("w2b", [D, D], bf16, kind="ExternalInput")
    t0_d = nc.dram_tensor("table0", [NC, ncp, D], bf16, kind="ExternalInput")
    idx_d = nc.dram_tensor("idxh", [P, NQ, nchmax, CI // 16], i16,
                           kind="ExternalInput")
    sidd_d = nc.dram_tensor("sidd", [P, max(n_dve_gen, 1)], f32,
                            kind="ExternalInput")
    sida_d = nc.dram_tensor("sida", [P, max(n_act_gen, 1)], f32,
                            kind="ExternalInput")
    lhsm_d = nc.dram_tensor("lhsm", [n_dma_ch, P, LCH, P], bf16,
                            kind="ExternalInput")
    ynm0_d = nc.dram_tensor("ynm0", [P, ntile, D], bf16, kind="ExternalInput")
    dinv2_d = nc.dram_tensor("dinv2", [ntile, P], f32, kind="ExternalInput")
    dinvc_d = nc.dram_tensor("dinvc", [ntile, P], f32, kind="ExternalInput")
    gmask_d = nc.dram_tensor("gmask", [ntile, P, NG], bf16, kind="ExternalInput")
    ident_d = nc.dram_tensor("ident", [P, P], bf16, kind="ExternalInput")
    iota_d = nc.dram_tensor("iota", [P, P], bf16, kind="ExternalInput")
    iotaf_d = nc.dram_tensor("iotaf", [P, P], f32, kind="ExternalInput")
    gsum_o = nc.dram_tensor("gsum", [NG, D], f32, kind="ExternalOutput")
    gmax_o = nc.dram_tensor("gmax", [P, NG], f32, kind="ExternalOutput")
    ystage = nc.dram_tensor("ystage", [ncp, D], bf16)
    tables = [t0_d] + [nc.dram_tensor(f"table{l}", [NC, ncp, D], bf16,
                                      addr_space="Shared") for l in (1, 2)]

    from contextlib import ExitStack
    es = ExitStack()
    block = es.enter_context(nc.Block())
    sb = lambda *a: es.enter_context(nc.sbuf_tensor(*a))
    ps = lambda *a: es.enter_context(nc.psum_tensor(*a))
    sem = lambda n: es.enter_context(nc.semaphore(n))

    msgs = sb("msgs", [P, NQ, NBUFC, CB, D], bf16)
    idx_sb = sb("idx_sb", [P, NQ, nchmax, CI // 16], i16)
    sidd_sb = sb("sidd_sb", [P, max(n_dve_gen, 1)], f32)
    sida_sb = sb("sida_sb", [P, max(n_act_gen, 1)], f32)
    lhsd = sb("lhsd", [P, NLD, P], bf16)
    lhsa = sb("lhsa", [P, NLA, P], bf16)
    lhsm = sb("lhsm_sb", [P, NLM, P], bf16)
    tmp_sb = sb("tmp_sb", [P, P], bf16)
    vnm = sb("vnm", [P, ntile, D], bf16)
    vfm = sb("vfm", [P, ntile, P], bf16)
    ynm = sb("ynm", [P, ntile, D], bf16)
    w_sb = sb("w_sb", [P, 2, D], bf16)
    dinv2_sb = sb("dinv2_sb", [P, ntile], f32)
    dinvc_sb = sb("dinvc_sb", [P, ntile], f32)
    gmask_sb = sb("gmask_sb", [P, ntile, NG], bf16)
    ident_sb = sb("ident_sb", [P, P], bf16)
    iota_sb = sb("iota_sb", [P, P], bf16)
    iotaf_sb = sb("iotaf_sb", [P, P], f32)
    gmax_sb = sb("gmax_sb", [P, NG], f32)
    gsum_sb = sb("gsum_sb", [NG, D], f32)
    ps_red = ps("ps_red", [P, 4, 512], f32)
    ps_aux = ps("ps_aux", [P, 2, 1024], bf16)
    ps_gs = ps("ps_gs", [NG, D], f32)
    sdma = sem("sdma")
    gsem = [sem(f"g{q}") for q in range(NQ)]
    pe_s = sem("pe")
    act_s = sem("act")
    dve_s = sem("dve")
    cc_s = sem("cc")

    if True:

        @block.sync
        def _(s):
            _anc = nc.allow_non_contiguous_dma(reason="one-time small loads")
            _anc.__enter__()
            s.dma_start(w_sb[:, 0, :], w1_d[:]).then_inc(sdma, 16)
            s.dma_start(w_sb[:, 1, :], w2_d[:]).then_inc(sdma, 16)
            s.dma_start(dinv2_sb[:], dinv2_d[:].rearrange("t p -> p t")).then_inc(sdma, 16)
            s.dma_start(dinvc_sb[:], dinvc_d[:].rearrange("t p -> p t")).then_inc(sdma, 16)
            s.dma_start(gmask_sb[:], gmask_d[:].rearrange("t p g -> p t g")).then_inc(sdma, 16)
            s.dma_start(ident_sb[:], ident_d[:]).then_inc(sdma, 16)
            s.dma_start(iota_sb[:], iota_d[:]).then_inc(sdma, 16)
            s.dma_start(iotaf_sb[:], iotaf_d[:]).then_inc(sdma, 16)
            s.dma_start(idx_sb[:], idx_d[:]).then_inc(sdma, 16)
            s.dma_start(sidd_sb[:], sidd_d[:]).then_inc(sdma, 16)
            s.dma_start(sida_sb[:], sida_d[:]).then_inc(sdma, 16)
            s.dma_start(ynm[:], ynm0_d[:]).then_inc(sdma, 16)
            _anc.__exit__(None, None, None)
            for l in range(3):
                for j in range(n_dma_ch):
                    if (wp := chunk_wait.get((l, j))) is not None:
                        s.wait_ge(pe_s, wp)
                    cslot = (j * LCH) % NLM
                    s.dma_start(lhsm[:, cslot:cslot + LCH, :],
                                lhsm_d[j]).then_inc(sdma, 16)
                if l < 2:
                    s.wait_ge(dve_s, dve_num[(l, 'ysc', ntile - 1)])
                    s.dma_start(ystage[:].rearrange("(t p) f -> p t f", p=P),
                                ynm[:]).then_inc(sdma, 16)
            s.wait_ge(dve_s, dve_num[(2, 'gmax', NG - 1)])
            s.wait_ge(act_s, act_num['gsum'])
            s.dma_start(gsum_o[:], gsum_sb[:]).then_inc(sdma, 16)
            s.dma_start(gmax_o[:], gmax_sb[:]).then_inc(sdma, 16)
            s.wait_ge(sdma, SD_TOTAL)

        @block.gpsimd
        def _(gp):
            gp.load_library(mlp)
            gp.wait_ge(sdma, SD_INIT)
            for l in range(3):
                if l > 0:
                    gp.wait_ge(cc_s, l)
                for ch in range(nchmax):
                    for q in range(NQ):
                        if ch >= nchunk_q[q]:
                            continue
                        wpe = gath_wait_pe[(l, q, ch)]
                        if wpe:
                            gp.wait_ge(pe_s, wpe)
                        gp.dma_gather(
                            msgs[:, q, ch % NBUFC, :, :],
                            tables[l][2 * q:2 * q + 2].rearrange(
                                "a b c -> (a b) c"),
                            idx_sb[:, q, ch, :],
                            CI, CI, D,
                            transpose=False,
                            single_packet=False,
                            queue_num=q,
                        ).then_inc(gsem[q], 16)
                if l < 2:
                    gp.wait_ge(sdma, SD_YST[l])
                    gp.collective_compute(
                        "AllGather", mybir.AluOpType.bypass,
                        ins=[ystage[:]], outs=[tables[l + 1][:]],
                        replica_groups=[list(range(NC))],
                    ).then_inc(cc_s, 1)

        @block.tensor
        def _(pe):
            pe.wait_ge(sdma, SD_INIT)
            for l in range(3):
                for kind, a in pe_seq(l):
                    if kind == 'ident':
                        t = a
                        if t >= 4:
                            pe.wait_ge(act_s, act_num[(l, 'ev', t - 4)])
                        if l < 2 and t >= 8:
                            pe.wait_ge(dve_s, dve_num[(l, 'ysc', t - 8)])
                        if l > 0:
                            tb = tail_tile(t % 4) if t < 4 else t
                            pe.wait_ge(dve_s, dve_num[(l - 1, 'ysc', tb)])
                        pe.matmul(out=ps_red[:, t % 4, :D], lhsT=ident_sb[:],
                                  rhs=ynm[:, t, :], start=True,
                                  stop=False).then_inc(pe_s, 1)
                    elif kind == 'agg':
                        i = a
                        t, q, w = ops[i]
                        ch = w // CB
                        pe.wait_ge(gsem[q], 16 * gath_n[(l, q, ch)])
                        e = gen_eng[i]
                        if e == 'dve':
                            pe.wait_ge(dve_s, dve_num[(l, 'gen', i)])
                            lhs_ap = lhsd[:, slot_dve[(l, i)], :]
                        elif e == 'act':
                            pe.wait_ge(act_s, act_num[(l, 'grl', i)])
                            lhs_ap = lhsa[:, slot_act[(l, i)], :]
                        else:
                            pe.wait_ge(sdma, sd_lhs[(l, dma_gi[i] // LCH)])
                            lhs_ap = lhsm[:, slot_dma[(l, i)], :]
                        pe.matmul(out=ps_red[:, t % 4, :D], lhsT=lhs_ap,
                                  rhs=msgs[:, q, ch % NBUFC, w % CB, :],
                                  start=False,
                                  stop=(i == tile_last_op[t])).then_inc(pe_s, 1)
                    elif kind == 'tr':
                        t = a
                        pe.wait_ge(act_s, act_num[(l, 'ev', t)])
                        if t >= 2:
                            pe.wait_ge(act_s, act_num[(l, 'vfm', t - 2)])
                        pe.transpose(out=ps_aux[:, t % 2, :P], in_=vnm[:, t, :],
                                     identity=ident_sb[:]).then_inc(pe_s, 1)
                    else:  # 'xw'
                        t = a
                        if l < 2:
                            pe.wait_ge(act_s, act_num[(l, 'vfm', t)])
                            pe.matmul(out=ps_red[:, t % 4, :D],
                                      lhsT=vfm[:, t, :], rhs=w_sb[:, l, :],
                                      start=True, stop=True).then_inc(pe_s, 1)
                        else:
                            pe.wait_ge(act_s, act_num[(2, 'ev', t)])
                            pe.matmul(out=ps_gs[:], lhsT=gmask_sb[:, t, :],
                                      rhs=vnm[:, t, :], start=(t == 0),
                                      stop=(t == ntile - 1)).then_inc(pe_s, 1)

        @block.scalar
        def _(a_):
            Act = mybir.ActivationFunctionType
            a_.wait_ge(sdma, SD_INIT)
            for l in range(3):
                for kind, a in act_seq(l):
                    if kind == 'gsq':
                        i = a
                        if (wp := wait_act.get((l, i))) is not None:
                            a_.wait_ge(pe_s, wp)
                        gi = act_gi[i]
                        a_.activation(tmp_sb[:], iota_sb[:], Act.Square,
                                      bias=sida_sb[:, gi:gi + 1]).then_inc(act_s, 1)
                    elif kind == 'grl':
                        i = a
                        a_.activation(lhsa[:, slot_act[(l, i)], :], tmp_sb[:],
                                      Act.Relu, scale=-1.0,
                                      bias=1.0).then_inc(act_s, 1)
                    elif kind == 'ev':
                        t = a
                        a_.wait_ge(pe_s, pe_num[(l, 'agg', tile_last_op[t])])
                        if l < 2:
                            a_.activation(vnm[:, t, :], ps_red[:, t % 4, :D],
                                          Act.Relu).then_inc(act_s, 1)
                        else:
                            a_.activation(vnm[:, t, :], ps_red[:, t % 4, :D],
                                          Act.Copy,
                                          scale=dinvc_sb[:, t:t + 1]
                                          ).then_inc(act_s, 1)
                    else:  # 'vfm'
                        t = a
                        a_.wait_ge(pe_s, pe_num[(l, 'tr', t)])
                        a_.activation(vfm[:, t, :], ps_aux[:, t % 2, :P],
                                      Act.Copy).then_inc(act_s, 1)
            a_.wait_ge(pe_s, pe_num[(2, 'xw', ntile - 1)])
            a_.activation(gsum_sb[:], ps_gs[:], Act.Copy).then_inc(act_s, 1)

        @block.vector
        def _(v):
            v.memset(gmax_sb[:], -1e30).then_inc(dve_s, 1)
            v.wait_ge(sdma, SD_INIT)
            for l in range(3):
                for kind, a in dve_seq(l):
                    if kind == 'gen':
                        i = a
                        if (wp := wait_dve.get((l, i))) is not None:
                            v.wait_ge(pe_s, wp)
                        gi = dve_gi[i]
                        v.tensor_scalar(out=lhsd[:, slot_dve[(l, i)], :],
                                        in0=iotaf_sb[:],
                                        scalar1=sidd_sb[:, gi:gi + 1],
                                        scalar2=None,
                                        op0=mybir.AluOpType.is_equal
                                        ).then_inc(dve_s, 1)
                    elif kind == 'ysc':
                        t = a
                        v.wait_ge(pe_s, pe_num[(l, 'xw', t)])
                        v.tensor_scalar(out=ynm[:, t, :],
                                        in0=ps_red[:, t % 4, :D],
                                        scalar1=dinv2_sb[:, t:t + 1],
                                        scalar2=None,
                                        op0=mybir.AluOpType.mult
                                        ).then_inc(dve_s, 1)
                    else:  # 'gmax'
                        g = a
                        a0, b0 = runs[g]
                        vf = vfm[:].rearrange("p t f -> p (t f)")
                        v.wait_ge(act_s, act_num[(2, 'vfm', (b0 - 1) // P)])
                        v.tensor_reduce(out=gmax_sb[:, g:g + 1],
                                        in_=vf[:, a0:b0],
                                        axis=mybir.AxisListType.X,
                                        op=mybir.AluOpType.max
                                        ).then_inc(dve_s, 1)

    es.close()
    nc.compile()
    return nc


def kernel(**inputs):
    node_type = np.asarray(inputs["node_type"]).astype(np.int64)
    ninv = np.asarray(inputs["num_inverted_predecessors"]).astype(np.int64)
    ei = np.asarray(inputs["edge_index"]).astype(np.int64)
    batch = np.asarray(inputs["batch"]).astype(np.int64)
    emb_type = np.asarray(inputs["emb_type"]).astype(np.float32)
    emb_inv = np.asarray(inputs["emb_inv"]).astype(np.float32)
    W0 = np.asarray(inputs["W0"]).astype(np.float32)
    W1 = np.asarray(inputs["W1"]).astype(np.float32)
    W2 = np.asarray(inputs["W2"]).astype(np.float32)

    tpl = _host_prep(node_type, ninv, ei[0], ei[1], batch, emb_type, emb_inv, W0)
    nc = _build(tpl)

    ident = np.eye(P, dtype=BF)
    iotaf = np.tile(np.arange(P, dtype=np.float32)[None, :], (P, 1))
    iota = iotaf.astype(BF)
    in_maps = []
    for c in range(NC):
        in_maps.append(dict(
            w1b=W1.astype(BF),
            w2b=W2.astype(BF),
            table0=tpl["table0"],
            idxh=tpl["data"][c]["idx"],
            sidd=tpl["data"][c]["sid_d"],
            sida=tpl["data"][c]["sid_a"],
            lhsm=tpl["data"][c]["lhsm"],
            ynm0=tpl["ynm0"][c],
            dinv2=tpl["dinv2"][c].reshape(tpl["ntile"], P),
            dinvc=tpl["dinvc"][c].reshape(tpl["ntile"], P),
            gmask=tpl["gmask"][c].reshape(tpl["ntile"], P, NG),
            ident=ident,
            iota=iota,
            iotaf=iotaf.astype(np.float32),
        ))
    import os
    trace = os.environ.get("BASS_KERNEL_TRACE", "0") == "1"
    if trace:
        sys.path.insert(0, "/root/problem/work")
        try:
            import axon_trace_patch  # noqa
        except Exception:
            trace = False
    res = run_bass_kernel_spmd(nc, in_maps, core_ids=list(range(NC)), trace=trace)
    kernel.last_exec_ns = res.exec_time_ns

    gsum = np.zeros((NG, D), dtype=np.float64)
    gmax = np.full((NG, D), -np.inf)
    for c in range(NC):
        gsum += res.results[c]["gsum"].astype(np.float64)
        gm = res.results[c]["gmax"].astype(np.float64).T   # [NG, P]
        pres = tpl["cells"][c] > 0
        gmax[pres] = np.maximum(gmax[pres], gm[pres])
    out = np.concatenate([gmax, gsum], axis=1).astype(np.float32)
    return (np.round(out * 1000.0) / 1000.0).astype(np.float32)


# revision 16
# speedup vs baseline: 1.8517x; 1.1997x over previous
"""Trainium2 Bass kernel: 3-layer GCN (AIGEncoder) + global max/sum readout.

8 NeuronCores SPMD, nodes sharded core = node % 8 (canonical per-graph cell
layout shared across cores). Per layer the edge aggregation is a gather of
bf16 y-rows (y = dinv * h @ W, replicated table in DRAM, AllGather between
layers) followed by TensorEngine one-hot segment-reduce matmuls into PSUM.

v2 structure: per-(core, quarter) packed gather streams in dst-canonical
order (per-tile length = max over cores, ~6% padding) on 4 SWDGE queues;
self-loop contribution via PE identity-matmul from SBUF-resident ynm; the
one-hot lhs matrices are generated on-chip (DVE is_equal against an iota
row, overflow share on the Act engine via Square/Relu) from per-partition
slot-id bytes; idx/slot-id data is SBUF-resident, loaded once.
"""
import sys

sys.path.insert(0, "/opt/trn_rl_repo")

import numpy as np
import ml_dtypes

import concourse.bacc as bacc
import concourse.bass as bass
import concourse.mybir as mybir
from concourse.bass_utils import run_bass_kernel_spmd
from concourse.library_config import mlp

P = 128
N = 100000
NG = 64
D = 128
NC = 8
NQ = 4                      # quarters (2 src cores each) = SWDGE queues
CI = 2048                   # rows per gather chunk
CB = CI // P                # windows per chunk
NBUFC = 3                   # msgs chunk ring depth per quarter
NLD = 16                    # DVE-generated lhs ring
NLA = 8                     # ACT-generated lhs ring
NLM = 32                    # DMA-loaded lhs ring (matrices)
LCH = 8                     # lhs matrices per DMA chunk
LAG = 6                     # tiles of lag for interleaved tr/xw/ysc
# per-op lhs source pattern (cycled): balance DVE/ACT gen vs DMA load
SRC_PATTERN = ["dve", "dma", "act", "dma", "dve", "dma", "dma", "act",
               "dve", "dma", "dma", "dma", "dve", "act", "dma", "dma",
               "dve", "dma", "act", "dma"]

BF = ml_dtypes.bfloat16


def _host_prep(node_type, ninv, src, dst, batch, emb_type, emb_inv, W0):
    deg = np.bincount(dst, minlength=N) + 1.0
    dinv = (1.0 / np.sqrt(deg)).astype(np.float64)

    cells = np.zeros((NC, NG), dtype=np.int64)
    for c in range(NC):
        cells[c] = np.bincount(batch[np.arange(c, N, NC)], minlength=NG)
    T = cells.max(axis=0)
    cell_start = np.concatenate([[0], np.cumsum(T)])
    ncanon = int(cell_start[-1])
    ntile = -(-ncanon // P)
    ncp = ntile * P

    canon_pos = np.full(N, -1, dtype=np.int64)
    for c in range(NC):
        nodes_c = np.arange(c, N, NC)
        b = batch[nodes_c]
        starts = np.searchsorted(b, np.arange(NG))
        rank = np.arange(len(nodes_c)) - starts[b]
        canon_pos[nodes_c] = cell_start[b] + rank
    trow = (np.arange(N) % NC % 2) * ncp + canon_pos  # row in quarter-pair table

    # ---- per-core edge streams (with pad-slot duplication of cell head) ----
    per_core = []
    for c in range(NC):
        sel = np.flatnonzero(dst % NC == c)
        s_, d_ = src[sel], dst[sel]
        slot = canon_pos[d_]
        qq = (s_ % NC) // 2
        tr = trow[s_]
        # pad slots duplicate the cell-head node's full segment
        ex_slot, ex_q, ex_tr = [], [], []
        order0 = np.lexsort((tr, slot, qq))
        sq_sorted = qq[order0] * ncp + slot[order0]
        for g in range(NG):
            if cells[c][g] == 0 or cells[c][g] == T[g]:
                continue
            f = cell_start[g]
            for q in range(4):
                lo = np.searchsorted(sq_sorted, q * ncp + f)
                hi = np.searchsorted(sq_sorted, q * ncp + f + 1)
                if hi == lo:
                    continue
                rows = tr[order0[lo:hi]]
                for j in range(cell_start[g] + cells[c][g], cell_start[g] + T[g]):
                    ex_slot.append(np.full(hi - lo, j))
                    ex_q.append(np.full(hi - lo, q))
                    ex_tr.append(rows)
        if ex_slot:
            slot = np.concatenate([slot, np.concatenate(ex_slot)])
            qq = np.concatenate([qq, np.concatenate(ex_q)])
            tr = np.concatenate([tr, np.concatenate(ex_tr)])
        per_core.append((slot.astype(np.int64), qq.astype(np.int64),
                         tr.astype(np.int64)))

    seglen = np.zeros((NC, ntile, NQ), dtype=np.int64)
    for c in range(NC):
        slot, qq, _ = per_core[c]
        cnt = np.bincount(slot * NQ + qq, minlength=ncp * NQ).reshape(ncp, NQ)
        seglen[c] = cnt.reshape(ntile, P, NQ).sum(axis=1)
    SEG = seglen.max(axis=0)                       # [ntile, NQ]
    off = np.zeros((NQ, ntile + 1), dtype=np.int64)
    for q in range(NQ):
        off[q, 1:] = np.cumsum(SEG[:, q])
    L = off[:, -1]
    Lpad = -(-L // CI) * CI
    nchunk_q = (Lpad // CI).astype(np.int64)
    nchmax = int(nchunk_q.max())

    # ---- shared op schedule: (t, q, w) agg ops ----
    ops = []                                       # (t, q, w)
    tile_first = np.zeros(ntile, dtype=np.int64)   # index into ops incl ident
    for t in range(ntile):
        tile_first[t] = len(ops)
        for q in range(NQ):
            a, b = int(off[q][t]), int(off[q][t + 1])
            if b > a:
                for w in range(a // P, (b - 1) // P + 1):
                    ops.append((t, q, w))
    n_agg = len(ops)
    # generator assignment (by agg-op index): dve / act / dma pattern
    gen_eng = [SRC_PATTERN[i % len(SRC_PATTERN)] for i in range(n_agg)]
    dve_gi = {}; act_gi = {}; dma_gi = {}
    for i, e in enumerate(gen_eng):
        if e == "dve":
            dve_gi[i] = len(dve_gi)
        elif e == "act":
            act_gi[i] = len(act_gi)
        else:
            dma_gi[i] = len(dma_gi)
    n_dve_gen, n_act_gen, n_dma = len(dve_gi), len(act_gi), len(dma_gi)

    # ---- per-core payloads ----
    data = []
    for c in range(NC):
        slot, qq, tr = per_core[c]
        order = np.lexsort((tr, slot, qq))
        slot, qq, tr = slot[order], qq[order], tr[order]
        key = qq * ntile + slot // P
        # rank within (q, tile) group
        grp_start = np.searchsorted(key, key, side="left")
        rank = np.arange(len(key)) - grp_start
        pos = off[qq, slot // P] + rank
        slotarr = np.full((NQ, int(Lpad.max())), -1, dtype=np.int32)
        trowarr = np.zeros((NQ, int(Lpad.max())), dtype=np.int64)
        slotarr[qq, pos] = slot
        trowarr[qq, pos] = tr

        # idx wrap: [P, NQ, nchmax, CI//16] int16
        idxh = np.zeros((P, NQ, nchmax, CI // 16), dtype=np.int16)
        for q in range(NQ):
            for ch in range(int(nchunk_q[q])):
                part = trowarr[q, ch * CI:(ch + 1) * CI].astype(np.int16)
                prs = part.reshape(CI // 16, 16)       # [128, 16]
                idxh[:, q, ch, :] = np.tile(prs.T, (8, 1))
        # sid payloads, split per generator engine; dma ops get full matrices
        sid_d = np.zeros((P, max(n_dve_gen, 1)), dtype=np.float32)
        sid_a = np.zeros((P, max(n_act_gen, 1)), dtype=np.float32)
        n_dma_ch = -(-max(n_dma, 1) // LCH)
        lhsm = np.zeros((n_dma_ch, P, LCH, P), dtype=BF)
        for i, (t, q, w) in enumerate(ops):
            sl = slotarr[q, w * P:(w + 1) * P].astype(np.int64)
            sc = sl - t * P
            sc[(sl < t * P) | (sl >= (t + 1) * P)] = -1
            if gen_eng[i] == "dve":
                sid_d[:, dve_gi[i]] = sc.astype(np.float32)
            elif gen_eng[i] == "act":
                sid_a[:, act_gi[i]] = (-sc).astype(np.float32)
            else:
                j = dma_gi[i]
                oh = np.zeros((P, P), dtype=BF)
                v = sc >= 0
                oh[np.arange(P)[v], sc[v].astype(np.int64)] = 1.0
                lhsm[j // LCH, :, j % LCH, :] = oh
        data.append(dict(idx=idxh, sid_d=sid_d, sid_a=sid_a, lhsm=lhsm))

    # ---- per-core canonical scalars / masks / y0 ----
    combo = (emb_type[:, None, :].astype(np.float64)
             + emb_inv[None, :, :].astype(np.float64)).reshape(12, D)
    cw = combo @ W0.astype(np.float64)             # [12, D]
    cid = node_type * 3 + ninv
    dinv2 = np.ones((NC, ncp), dtype=np.float32)
    dinvc = np.ones((NC, ncp), dtype=np.float32)
    gmask = np.zeros((NC, ncp, NG), dtype=BF)
    y0 = np.zeros((NC, ncp, D), dtype=np.float32)
    for c in range(NC):
        nodes_c = np.arange(c, N, NC)
        cp = canon_pos[nodes_c]
        dinv2[c, cp] = (dinv[nodes_c] ** 2).astype(np.float32)
        dinvc[c, cp] = dinv[nodes_c].astype(np.float32)
        gmask[c, cp, batch[nodes_c]] = 1.0
        y0[c, cp, :] = (cw[cid[nodes_c]] * dinv[nodes_c][:, None]).astype(np.float32)
        # pad slots mirror the cell head
        for g in range(NG):
            if cells[c][g] == 0:
                continue
            f = cell_start[g]
            for j in range(cell_start[g] + cells[c][g], cell_start[g] + T[g]):
                dinv2[c, j] = dinv2[c, f]
                dinvc[c, j] = dinvc[c, f]
                y0[c, j, :] = y0[c, f, :]
    table0 = y0.astype(BF)                          # [NC, ncp, D]
    ynm0 = np.zeros((NC, P, ntile, D), dtype=BF)
    for c in range(NC):
        ynm0[c] = table0[c].reshape(ntile, P, D).transpose(1, 0, 2)

    runs = [(int(cell_start[g]), int(cell_start[g + 1])) for g in range(NG)]
    return dict(ntile=ntile, ncp=ncp, ops=ops, gen_eng=gen_eng,
                dve_gi=dve_gi, act_gi=act_gi, dma_gi=dma_gi,
                n_dve_gen=n_dve_gen,
                n_act_gen=n_act_gen, n_dma=n_dma,
                nchunk_q=[int(x) for x in nchunk_q],
                nchmax=nchmax, data=data, dinv2=dinv2, dinvc=dinvc,
                gmask=gmask, table0=table0, ynm0=ynm0, runs=runs, cells=cells)


def _build(tpl):
    ntile, ncp = tpl["ntile"], tpl["ncp"]
    ops, gen_eng = tpl["ops"], tpl["gen_eng"]
    dve_gi, act_gi, dma_gi = tpl["dve_gi"], tpl["act_gi"], tpl["dma_gi"]
    n_dve_gen, n_act_gen, n_dma = tpl["n_dve_gen"], tpl["n_act_gen"], tpl["n_dma"]
    nchunk_q, nchmax = tpl["nchunk_q"], tpl["nchmax"]
    runs = tpl["runs"]
    n_agg = len(ops)
    n_dma_ch = -(-max(n_dma, 1) // LCH)
    dt = mybir.dt
    f32, bf16, i16 = dt.float32, dt.bfloat16, dt.int16

    tile_last_op = {}
    tile_ops = {}
    for i, (t, q, w) in enumerate(ops):
        tile_last_op[t] = i
        tile_ops.setdefault(t, []).append(i)
    for t in range(ntile):
        assert t in tile_ops, "tile without aggs unsupported"

    def tail_tile(b):
        t = ntile - 1
        while t % 4 != b:
            t -= 1
        return t

    # ---------- instruction sequences (shared by numbering and emission) ----
    def pe_seq(l):
        out = []
        for t in range(ntile):
            out.append(('ident', t))
            for i in tile_ops[t]:
                out.append(('agg', i))
            if t >= LAG:
                out.append(('tr', t - LAG))
                out.append(('xw', t - LAG))
        for t in range(ntile - LAG, ntile):
            out.append(('tr', t))
            out.append(('xw', t))
        return out

    def act_seq(l):
        out = []
        for t in range(ntile):
            for i in tile_ops[t]:
                if gen_eng[i] == 'act':
                    out.append(('gsq', i))
                    out.append(('grl', i))
            out.append(('ev', t))
            if t >= LAG:
                out.append(('vfm', t - LAG))
        for t in range(ntile - LAG, ntile):
            out.append(('vfm', t))
        return out

    def dve_seq(l):
        out = []
        for t in range(ntile):
            for i in tile_ops[t]:
                if gen_eng[i] == 'dve':
                    out.append(('gen', i))
            if l < 2 and t >= LAG:
                out.append(('ysc', t - LAG))
        if l < 2:
            for t in range(ntile - LAG, ntile):
                out.append(('ysc', t))
        else:
            for g in range(NG):
                out.append(('gmax', g))
        return out

    # ---------- numbering ----------
    pe_num, act_num, dve_num = {}, {}, {}
    k = 0
    for l in range(3):
        for rec in pe_seq(l):
            k += 1
            pe_num[(l,) + rec] = k
    k = 0
    for l in range(3):
        for rec in act_seq(l):
            k += 1
            act_num[(l,) + rec] = k
    k += 1
    act_num['gsum'] = k
    k = 1  # memset
    for l in range(3):
        for rec in dve_seq(l):
            k += 1
            dve_num[(l,) + rec] = k

    # gather numbering (cumulative per quarter sem)
    gath_n = {}
    cnt_q = [0] * NQ
    for l in range(3):
        for ch in range(nchmax):
            for q in range(NQ):
                if ch < nchunk_q[q]:
                    cnt_q[q] += 1
                    gath_n[(l, q, ch)] = cnt_q[q]

    # msgs buffer recycle
    last_pe_of_chunk = {}
    for l in range(3):
        for i, (t, q, w) in enumerate(ops):
            last_pe_of_chunk[(l, q, w // CB)] = pe_num[(l, 'agg', i)]
    gath_wait_pe = {}
    prev_use = {}
    for l in range(3):
        for ch in range(nchmax):
            for q in range(NQ):
                if ch >= nchunk_q[q]:
                    continue
                par = ch % NBUFC
                gath_wait_pe[(l, q, ch)] = prev_use.get((q, par), 0)
                prev_use[(q, par)] = last_pe_of_chunk.get(
                    (l, q, ch), prev_use.get((q, par), 0))

    # generator/DMA lhs rings: slot + writer-wait (pe# of previous occupant)
    def ring_plan(idx_map, n_per_layer, depth):
        slot_of, wait_of = {}, {}
        prev = {}
        for l in range(3):
            for i in sorted(idx_map, key=idx_map.get):
                s = idx_map[i] % depth
                slot_of[(l, i)] = s
                if (ps := prev.get(s)) is not None:
                    wait_of[(l, i)] = pe_num[(ps[0], 'agg', ps[1])]
                prev[s] = (l, i)
        return slot_of, wait_of

    slot_dve, wait_dve = ring_plan(dve_gi, n_dve_gen, NLD)
    slot_act, wait_act = ring_plan(act_gi, n_act_gen, NLA)
    slot_dma, _ = ring_plan(dma_gi, n_dma, NLM)
    # lhs DMA chunks: (l, j) -> wait pe# for ring reuse
    inv_dma = {v: k2 for k2, v in dma_gi.items()}
    chunk_wait = {}
    prev_chunk = {}
    for l in range(3):
        for j in range(n_dma_ch):
            cslot = (j * LCH) % NLM
            if (pc := prev_chunk.get(cslot)) is not None:
                lp, jp = pc
                last = min(jp * LCH + LCH, n_dma) - 1
                chunk_wait[(l, j)] = pe_num[(lp, 'agg', inv_dma[last])]
            prev_chunk[cslot] = (l, j)

    # sync sdma numbering
    N_INIT = 12
    sd = N_INIT
    sd_lhs = {}
    SD_YST = {}
    for l in range(3):
        for j in range(n_dma_ch):
            sd += 1
            sd_lhs[(l, j)] = sd * 16
        if l < 2:
            sd += 1
            SD_YST[l] = sd * 16
    SD_TOTAL = (sd + 2) * 16
    SD_INIT = N_INIT * 16

    # ---------- build ----------
    nc = bacc.Bacc("TRN2", debug=False, num_swdge_queues=4, num_devices=NC)
    w1_d = nc.dram_tensor("w1b", [D, D], bf16, kind="ExternalInput")
    w2_d = nc.dram_tensor("w2b", [D, D], bf16, kind="ExternalInput")
    t0_d = nc.dram_tensor("table0", [NC, ncp, D], bf16, kind="ExternalInput")
    idx_d = nc.dram_tensor("idxh", [P, NQ, nchmax, CI // 16], i16,
                           kind="ExternalInput")
    sidd_d = nc.dram_tensor("sidd", [P, max(n_dve_gen, 1)], f32,
                            kind="ExternalInput")
    sida_d = nc.dram_tensor("sida", [P, max(n_act_gen, 1)], f32,
                            kind="ExternalInput")
    lhsm_d = nc.dram_tensor("lhsm", [n_dma_ch, P, LCH, P], bf16,
                            kind="ExternalInput")
    ynm0_d = nc.dram_tensor("ynm0", [P, ntile, D], bf16, kind="ExternalInput")
    dinv2_d = nc.dram_tensor("dinv2", [ntile, P], f32, kind="ExternalInput")
    dinvc_d = nc.dram_tensor("dinvc", [ntile, P], f32, kind="ExternalInput")
    gmask_d = nc.dram_tensor("gmask", [ntile, P, NG], bf16, kind="ExternalInput")
    ident_d = nc.dram_tensor("ident", [P, P], bf16, kind="ExternalInput")
    iota_d = nc.dram_tensor("iota", [P, P], bf16, kind="ExternalInput")
    iotaf_d = nc.dram_tensor("iotaf", [P, P], f32, kind="ExternalInput")
    gsum_o = nc.dram_tensor("gsum", [NG, D], f32, kind="ExternalOutput")
    gmax_o = nc.dram_tensor("gmax", [P, NG], f32, kind="ExternalOutput")
    ystage = nc.dram_tensor("ystage", [ncp, D], bf16)
    tables = [t0_d] + [nc.dram_tensor(f"table{l}", [NC, ncp, D], bf16,
                                      addr_space="Shared") for l in (1, 2)]

    from contextlib import ExitStack
    es = ExitStack()
    block = es.enter_context(nc.Block())
    sb = lambda *a: es.enter_context(nc.sbuf_tensor(*a))
    ps = lambda *a: es.enter_context(nc.psum_tensor(*a))
    sem = lambda n: es.enter_context(nc.semaphore(n))

    msgs = sb("msgs", [P, NQ, NBUFC, CB, D], bf16)
    idx_sb = sb("idx_sb", [P, NQ, nchmax, CI // 16], i16)
    sidd_sb = sb("sidd_sb", [P, max(n_dve_gen, 1)], f32)
    sida_sb = sb("sida_sb", [P, max(n_act_gen, 1)], f32)
    lhsd = sb("lhsd", [P, NLD, P], bf16)
    lhsa = sb("lhsa", [P, NLA, P], bf16)
    lhsm = sb("lhsm_sb", [P, NLM, P], bf16)
    tmp_sb = sb("tmp_sb", [P, P], bf16)
    vnm = sb("vnm", [P, ntile, D], bf16)
    vfm = sb("vfm", [P, ntile, P], bf16)
    ynm = sb("ynm", [P, ntile, D], bf16)
    w_sb = sb("w_sb", [P, 2, D], bf16)
    dinv2_sb = sb("dinv2_sb", [P, ntile], f32)
    dinvc_sb = sb("dinvc_sb", [P, ntile], f32)
    gmask_sb = sb("gmask_sb", [P, ntile, NG], bf16)
    ident_sb = sb("ident_sb", [P, P], bf16)
    iota_sb = sb("iota_sb", [P, P], bf16)
    iotaf_sb = sb("iotaf_sb", [P, P], f32)
    gmax_sb = sb("gmax_sb", [P, NG], f32)
    gsum_sb = sb("gsum_sb", [NG, D], f32)
    ps_red = ps("ps_red", [P, 4, 512], f32)
    ps_aux = ps("ps_aux", [P, 2, 1024], bf16)
    ps_gs = ps("ps_gs", [NG, D], f32)
    sdma = sem("sdma")
    gsem = [sem(f"g{q}") for q in range(NQ)]
    pe_s = sem("pe")
    act_s = sem("act")
    dve_s = sem("dve")
    cc_s = sem("cc")

    if True:

        @block.sync
        def _(s):
            _anc = nc.allow_non_contiguous_dma(reason="one-time small loads")
            _anc.__enter__()
            s.dma_start(w_sb[:, 0, :], w1_d[:]).then_inc(sdma, 16)
            s.dma_start(w_sb[:, 1, :], w2_d[:]).then_inc(sdma, 16)
            s.dma_start(dinv2_sb[:], dinv2_d[:].rearrange("t p -> p t")).then_inc(sdma, 16)
            s.dma_start(dinvc_sb[:], dinvc_d[:].rearrange("t p -> p t")).then_inc(sdma, 16)
            s.dma_start(gmask_sb[:], gmask_d[:].rearrange("t p g -> p t g")).then_inc(sdma, 16)
            s.dma_start(ident_sb[:], ident_d[:]).then_inc(sdma, 16)
            s.dma_start(iota_sb[:], iota_d[:]).then_inc(sdma, 16)
            s.dma_start(iotaf_sb[:], iotaf_d[:]).then_inc(sdma, 16)
            s.dma_start(idx_sb[:], idx_d[:]).then_inc(sdma, 16)
            s.dma_start(sidd_sb[:], sidd_d[:]).then_inc(sdma, 16)
            s.dma_start(sida_sb[:], sida_d[:]).then_inc(sdma, 16)
            s.dma_start(ynm[:], ynm0_d[:]).then_inc(sdma, 16)
            _anc.__exit__(None, None, None)
            for l in range(3):
                for j in range(n_dma_ch):
                    if (wp := chunk_wait.get((l, j))) is not None:
                        s.wait_ge(pe_s, wp)
                    cslot = (j * LCH) % NLM
                    s.dma_start(lhsm[:, cslot:cslot + LCH, :],
                                lhsm_d[j]).then_inc(sdma, 16)
                if l < 2:
                    s.wait_ge(dve_s, dve_num[(l, 'ysc', ntile - 1)])
                    s.dma_start(ystage[:].rearrange("(t p) f -> p t f", p=P),
                                ynm[:]).then_inc(sdma, 16)
            s.wait_ge(dve_s, dve_num[(2, 'gmax', NG - 1)])
            s.wait_ge(act_s, act_num['gsum'])
            s.dma_start(gsum_o[:], gsum_sb[:]).then_inc(sdma, 16)
            s.dma_start(gmax_o[:], gmax_sb[:]).then_inc(sdma, 16)
            s.wait_ge(sdma, SD_TOTAL)

        @block.gpsimd
        def _(gp):
            gp.load_library(mlp)
            gp.wait_ge(sdma, SD_INIT)
            for l in range(3):
                if l > 0:
                    gp.wait_ge(cc_s, l)
                for ch in range(nchmax):
                    for q in range(NQ):
                        if ch >= nchunk_q[q]:
                            continue
                        wpe = gath_wait_pe[(l, q, ch)]
                        if wpe:
                            gp.wait_ge(pe_s, wpe)
                        gp.dma_gather(
                            msgs[:, q, ch % NBUFC, :, :],
                            tables[l][2 * q:2 * q + 2].rearrange(
                                "a b c -> (a b) c"),
                            idx_sb[:, q, ch, :],
                            CI, CI, D,
                            transpose=False,
                            single_packet=False,
                            queue_num=q,
                        ).then_inc(gsem[q], 16)
                if l < 2:
                    gp.wait_ge(sdma, SD_YST[l])
                    gp.collective_compute(
                        "AllGather", mybir.AluOpType.bypass,
                        ins=[ystage[:]], outs=[tables[l + 1][:]],
                        replica_groups=[list(range(NC))],
                    ).then_inc(cc_s, 1)

        @block.tensor
        def _(pe):
            pe.wait_ge(sdma, SD_INIT)
            for l in range(3):
                for kind, a in pe_seq(l):
                    if kind == 'ident':
                        t = a
                        if t >= 4:
                            pe.wait_ge(act_s, act_num[(l, 'ev', t - 4)])
                        if l < 2 and t >= 8:
                            pe.wait_ge(dve_s, dve_num[(l, 'ysc', t - 8)])
                        if l > 0:
                            tb = tail_tile(t % 4) if t < 4 else t
                            pe.wait_ge(dve_s, dve_num[(l - 1, 'ysc', tb)])
                        pe.matmul(out=ps_red[:, t % 4, :D], lhsT=ident_sb[:],
                                  rhs=ynm[:, t, :], start=True,
                                  stop=False).then_inc(pe_s, 1)
                    elif kind == 'agg':
                        i = a
                        t, q, w = ops[i]
                        ch = w // CB
                        pe.wait_ge(gsem[q], 16 * gath_n[(l, q, ch)])
                        e = gen_eng[i]
                        if e == 'dve':
                            pe.wait_ge(dve_s, dve_num[(l, 'gen', i)])
                            lhs_ap = lhsd[:, slot_dve[(l, i)], :]
                        elif e == 'act':
                            pe.wait_ge(act_s, act_num[(l, 'grl', i)])
                            lhs_ap = lhsa[:, slot_act[(l, i)], :]
                        else:
                            pe.wait_ge(sdma, sd_lhs[(l, dma_gi[i] // LCH)])
                            lhs_ap = lhsm[:, slot_dma[(l, i)], :]
                        pe.matmul(out=ps_red[:, t % 4, :D], lhsT=lhs_ap,
                                  rhs=msgs[:, q, ch % NBUFC, w % CB, :],
                                  start=False,
                                  stop=(i == tile_last_op[t])).then_inc(pe_s, 1)
                    elif kind == 'tr':
                        t = a
                        pe.wait_ge(act_s, act_num[(l, 'ev', t)])
                        if t >= 2:
                            pe.wait_ge(act_s, act_num[(l, 'vfm', t - 2)])
                        pe.transpose(out=ps_aux[:, t % 2, :P], in_=vnm[:, t, :],
                                     identity=ident_sb[:]).then_inc(pe_s, 1)
                    else:  # 'xw'
                        t = a
                        if l < 2:
                            pe.wait_ge(act_s, act_num[(l, 'vfm', t)])
                            pe.matmul(out=ps_red[:, t % 4, :D],
                                      lhsT=vfm[:, t, :], rhs=w_sb[:, l, :],
                                      start=True, stop=True).then_inc(pe_s, 1)
                        else:
                            pe.wait_ge(act_s, act_num[(2, 'ev', t)])
                            pe.matmul(out=ps_gs[:], lhsT=gmask_sb[:, t, :],
                                      rhs=vnm[:, t, :], start=(t == 0),
                                      stop=(t == ntile - 1)).then_inc(pe_s, 1)

        @block.scalar
        def _(a_):
            Act = mybir.ActivationFunctionType
            a_.wait_ge(sdma, SD_INIT)
            for l in range(3):
                for kind, a in act_seq(l):
                    if kind == 'gsq':
                        i = a
                        if (wp := wait_act.get((l, i))) is not None:
                            a_.wait_ge(pe_s, wp)
                        gi = act_gi[i]
                        a_.activation(tmp_sb[:], iota_sb[:], Act.Square,
                                      bias=sida_sb[:, gi:gi + 1]).then_inc(act_s, 1)
                    elif kind == 'grl':
                        i = a
                        a_.activation(lhsa[:, slot_act[(l, i)], :], tmp_sb[:],
                                      Act.Relu, scale=-1.0,
                                      bias=1.0).then_inc(act_s, 1)
                    elif kind == 'ev':
                        t = a
                        a_.wait_ge(pe_s, pe_num[(l, 'agg', tile_last_op[t])])
                        if l < 2:
                            a_.activation(vnm[:, t, :], ps_red[:, t % 4, :D],
                                          Act.Relu).then_inc(act_s, 1)
                        else:
                            a_.activation(vnm[:, t, :], ps_red[:, t % 4, :D],
                                          Act.Copy,
                                          scale=dinvc_sb[:, t:t + 1]
                                          ).then_inc(act_s, 1)
                    else:  # 'vfm'
                        t = a
                        a_.wait_ge(pe_s, pe_num[(l, 'tr', t)])
                        a_.activation(vfm[:, t, :], ps_aux[:, t % 2, :P],
                                      Act.Copy).then_inc(act_s, 1)
            a_.wait_ge(pe_s, pe_num[(2, 'xw', ntile - 1)])
            a_.activation(gsum_sb[:], ps_gs[:], Act.Copy).then_inc(act_s, 1)

        @block.vector
        def _(v):
            v.memset(gmax_sb[:], -1e30).then_inc(dve_s, 1)
            v.wait_ge(sdma, SD_INIT)
            for l in range(3):
                for kind, a in dve_seq(l):
                    if kind == 'gen':
                        i = a
                        if (wp := wait_dve.get((l, i))) is not None:
                            v.wait_ge(pe_s, wp)
                        gi = dve_gi[i]
                        v.tensor_scalar(out=lhsd[:, slot_dve[(l, i)], :],
                                        in0=iota_sb[:],
                                        scalar1=sidd_sb[:, gi:gi + 1],
                                        scalar2=None,
                                        op0=mybir.AluOpType.is_equal
                                        ).then_inc(dve_s, 1)
                    elif kind == 'ysc':
                        t = a
                        v.wait_ge(pe_s, pe_num[(l, 'xw', t)])
                        v.tensor_scalar(out=ynm[:, t, :],
                                        in0=ps_red[:, t % 4, :D],
                                        scalar1=dinv2_sb[:, t:t + 1],
                                        scalar2=None,
                                        op0=mybir.AluOpType.mult
                                        ).then_inc(dve_s, 1)
                    else:  # 'gmax'
                        g = a
                        a0, b0 = runs[g]
                        vf = vfm[:].rearrange("p t f -> p (t f)")
                        v.wait_ge(act_s, act_num[(2, 'vfm', (b0 - 1) // P)])
                        v.tensor_reduce(out=gmax_sb[:, g:g + 1],
                                        in_=vf[:, a0:b0],
                                        axis=mybir.AxisListType.X,
                                        op=mybir.AluOpType.max
                                        ).then_inc(dve_s, 1)

    es.close()
    nc.compile()
    return nc


def kernel(**inputs):
    node_type = np.asarray(inputs["node_type"]).astype(np.int64)
    ninv = np.asarray(inputs["num_inverted_predecessors"]).astype(np.int64)
    ei = np.asarray(inputs["edge_index"]).astype(np.int64)
    batch = np.asarray(inputs["batch"]).astype(np.int64)
    emb_type = np.asarray(inputs["emb_type"]).astype(np.float32)
    emb_inv = np.asarray(inputs["emb_inv"]).astype(np.float32)
    W0 = np.asarray(inputs["W0"]).astype(np.float32)
    W1 = np.asarray(inputs["W1"]).astype(np.float32)
    W2 = np.asarray(inputs["W2"]).astype(np.float32)

    tpl = _host_prep(node_type, ninv, ei[0], ei[1], batch, emb_type, emb_inv, W0)
    nc = _build(tpl)

    ident = np.eye(P, dtype=BF)
    iotaf = np.tile(np.arange(P, dtype=np.float32)[None, :], (P, 1))
    iota = iotaf.astype(BF)
    in_maps = []
    for c in range(NC):
        in_maps.append(dict(
            w1b=W1.astype(BF),
            w2b=W2.astype(BF),
            table0=tpl["table0"],
            idxh=tpl["data"][c]["idx"],
            sidd=tpl["data"][c]["sid_d"],
            sida=tpl["data"][c]["sid_a"],
            lhsm=tpl["data"][c]["lhsm"],
            ynm0=tpl["ynm0"][c],
            dinv2=tpl["dinv2"][c].reshape(tpl["ntile"], P),
            dinvc=tpl["dinvc"][c].reshape(tpl["ntile"], P),
            gmask=tpl["gmask"][c].reshape(tpl["ntile"], P, NG),
            ident=ident,
            iota=iota,
            iotaf=iotaf.astype(np.float32),
        ))
    import os
    trace = os.environ.get("BASS_KERNEL_TRACE", "0") == "1"
    if trace:
        sys.path.insert(0, "/root/problem/work")
        try:
            import axon_trace_patch  # noqa
        except Exception:
            trace = False
    res = run_bass_kernel_spmd(nc, in_maps, core_ids=list(range(NC)), trace=trace)
    kernel.last_exec_ns = res.exec_time_ns

    gsum = np.zeros((NG, D), dtype=np.float64)
    gmax = np.full((NG, D), -np.inf)
    for c in range(NC):
        gsum += res.results[c]["gsum"].astype(np.float64)
        gm = res.results[c]["gmax"].astype(np.float64).T   # [NG, P]
        pres = tpl["cells"][c] > 0
        gmax[pres] = np.maximum(gmax[pres], gm[pres])
    out = np.concatenate([gmax, gsum], axis=1).astype(np.float32)
    return (np.round(out * 1000.0) / 1000.0).astype(np.float32)


# revision 17
# speedup vs baseline: 2.3271x; 1.2567x over previous
"""Trainium2 Bass kernel: 3-layer GCN (AIGEncoder) + global max/sum readout.

8 NeuronCores SPMD, nodes sharded core = node % 8 (canonical per-graph cell
layout shared across cores). Per layer the edge aggregation is a gather of
bf16 y-rows (y = dinv * h @ W, replicated table in DRAM, AllGather between
layers) followed by TensorEngine one-hot segment-reduce matmuls into PSUM.

v2 structure: per-(core, quarter) packed gather streams in dst-canonical
order (per-tile length = max over cores, ~6% padding) on 4 SWDGE queues;
self-loop contribution via PE identity-matmul from SBUF-resident ynm; the
one-hot lhs matrices are generated on-chip (DVE is_equal against an iota
row, overflow share on the Act engine via Square/Relu) from per-partition
slot-id bytes; idx/slot-id data is SBUF-resident, loaded once.
"""
import sys

sys.path.insert(0, "/opt/trn_rl_repo")

import numpy as np
import ml_dtypes

import concourse.bacc as bacc
import concourse.bass as bass
import concourse.mybir as mybir
from concourse.bass_utils import run_bass_kernel_spmd
from concourse.library_config import mlp

P = 128
N = 100000
NG = 64
D = 128
NC = 8
NQ = 4                      # quarters (2 src cores each) = SWDGE queues
CI = 2048                   # rows per gather chunk
CB = CI // P                # windows per chunk
NBUFC = 3                   # msgs chunk ring depth per quarter
NLD = 16                    # DVE-generated lhs ring
NLA = 8                     # ACT-generated lhs ring
NLM = 32                    # DMA-loaded lhs ring (matrices)
LCH = 8                     # lhs matrices per DMA chunk
LAG = 6                     # tiles of lag for interleaved tr/xw/ysc
# per-op lhs source pattern (cycled): balance DVE/ACT gen vs DMA load
SRC_PATTERN = ["dve", "dma", "act", "dma", "dve", "dma", "dma", "act",
               "dve", "dma", "dma", "dma", "dve", "act", "dma", "dma",
               "dve", "dma", "act", "dma"]

BF = ml_dtypes.bfloat16


def _host_prep(node_type, ninv, src, dst, batch, emb_type, emb_inv, W0):
    deg = np.bincount(dst, minlength=N) + 1.0
    dinv = (1.0 / np.sqrt(deg)).astype(np.float64)

    cells = np.zeros((NC, NG), dtype=np.int64)
    for c in range(NC):
        cells[c] = np.bincount(batch[np.arange(c, N, NC)], minlength=NG)
    T = cells.max(axis=0)
    cell_start = np.concatenate([[0], np.cumsum(T)])
    ncanon = int(cell_start[-1])
    ntile = -(-ncanon // P)
    ncp = ntile * P

    canon_pos = np.full(N, -1, dtype=np.int64)
    for c in range(NC):
        nodes_c = np.arange(c, N, NC)
        b = batch[nodes_c]
        starts = np.searchsorted(b, np.arange(NG))
        rank = np.arange(len(nodes_c)) - starts[b]
        canon_pos[nodes_c] = cell_start[b] + rank
    trow = (np.arange(N) % NC % 2) * ncp + canon_pos  # row in quarter-pair table

    # ---- per-core edge streams (with pad-slot duplication of cell head) ----
    per_core = []
    for c in range(NC):
        sel = np.flatnonzero(dst % NC == c)
        s_, d_ = src[sel], dst[sel]
        slot = canon_pos[d_]
        qq = (s_ % NC) // 2
        tr = trow[s_]
        # pad slots duplicate the cell-head node's full segment
        ex_slot, ex_q, ex_tr = [], [], []
        order0 = np.lexsort((tr, slot, qq))
        sq_sorted = qq[order0] * ncp + slot[order0]
        for g in range(NG):
            if cells[c][g] == 0 or cells[c][g] == T[g]:
                continue
            f = cell_start[g]
            for q in range(4):
                lo = np.searchsorted(sq_sorted, q * ncp + f)
                hi = np.searchsorted(sq_sorted, q * ncp + f + 1)
                if hi == lo:
                    continue
                rows = tr[order0[lo:hi]]
                for j in range(cell_start[g] + cells[c][g], cell_start[g] + T[g]):
                    ex_slot.append(np.full(hi - lo, j))
                    ex_q.append(np.full(hi - lo, q))
                    ex_tr.append(rows)
        if ex_slot:
            slot = np.concatenate([slot, np.concatenate(ex_slot)])
            qq = np.concatenate([qq, np.concatenate(ex_q)])
            tr = np.concatenate([tr, np.concatenate(ex_tr)])
        per_core.append((slot.astype(np.int64), qq.astype(np.int64),
                         tr.astype(np.int64)))

    seglen = np.zeros((NC, ntile, NQ), dtype=np.int64)
    for c in range(NC):
        slot, qq, _ = per_core[c]
        cnt = np.bincount(slot * NQ + qq, minlength=ncp * NQ).reshape(ncp, NQ)
        seglen[c] = cnt.reshape(ntile, P, NQ).sum(axis=1)
    SEG = seglen.max(axis=0)                       # [ntile, NQ]
    off = np.zeros((NQ, ntile + 1), dtype=np.int64)
    for q in range(NQ):
        off[q, 1:] = np.cumsum(SEG[:, q])
    L = off[:, -1]
    Lpad = -(-L // CI) * CI
    nchunk_q = (Lpad // CI).astype(np.int64)
    nchmax = int(nchunk_q.max())

    # ---- shared op schedule: (t, q, w) agg ops ----
    ops = []                                       # (t, q, w)
    tile_first = np.zeros(ntile, dtype=np.int64)   # index into ops incl ident
    for t in range(ntile):
        tile_first[t] = len(ops)
        for q in range(NQ):
            a, b = int(off[q][t]), int(off[q][t + 1])
            if b > a:
                for w in range(a // P, (b - 1) // P + 1):
                    ops.append((t, q, w))
    n_agg = len(ops)
    # generator assignment (by agg-op index): dve / act / dma pattern
    gen_eng = [SRC_PATTERN[i % len(SRC_PATTERN)] for i in range(n_agg)]
    dve_gi = {}; act_gi = {}; dma_gi = {}
    for i, e in enumerate(gen_eng):
        if e == "dve":
            dve_gi[i] = len(dve_gi)
        elif e == "act":
            act_gi[i] = len(act_gi)
        else:
            dma_gi[i] = len(dma_gi)
    n_dve_gen, n_act_gen, n_dma = len(dve_gi), len(act_gi), len(dma_gi)

    # ---- per-core payloads ----
    data = []
    for c in range(NC):
        slot, qq, tr = per_core[c]
        order = np.lexsort((tr, slot, qq))
        slot, qq, tr = slot[order], qq[order], tr[order]
        key = qq * ntile + slot // P
        # rank within (q, tile) group
        grp_start = np.searchsorted(key, key, side="left")
        rank = np.arange(len(key)) - grp_start
        pos = off[qq, slot // P] + rank
        slotarr = np.full((NQ, int(Lpad.max())), -1, dtype=np.int32)
        trowarr = np.zeros((NQ, int(Lpad.max())), dtype=np.int64)
        slotarr[qq, pos] = slot
        trowarr[qq, pos] = tr

        # idx wrap: [P, NQ, nchmax, CI//16] int16
        idxh = np.zeros((P, NQ, nchmax, CI // 16), dtype=np.int16)
        for q in range(NQ):
            for ch in range(int(nchunk_q[q])):
                part = trowarr[q, ch * CI:(ch + 1) * CI].astype(np.int16)
                prs = part.reshape(CI // 16, 16)       # [128, 16]
                idxh[:, q, ch, :] = np.tile(prs.T, (8, 1))
        # sid payloads, split per generator engine; dma ops get full matrices
        sid_d = np.zeros((P, max(n_dve_gen, 1)), dtype=np.float32)
        sid_a = np.zeros((P, max(n_act_gen, 1)), dtype=np.float32)
        n_dma_ch = -(-max(n_dma, 1) // LCH)
        lhsm = np.zeros((n_dma_ch, P, LCH, P), dtype=BF)
        for i, (t, q, w) in enumerate(ops):
            sl = slotarr[q, w * P:(w + 1) * P].astype(np.int64)
            sc = sl - t * P
            sc[(sl < t * P) | (sl >= (t + 1) * P)] = -1
            if gen_eng[i] == "dve":
                sid_d[:, dve_gi[i]] = sc.astype(np.float32)
            elif gen_eng[i] == "act":
                sid_a[:, act_gi[i]] = (-sc).astype(np.float32)
            else:
                j = dma_gi[i]
                oh = np.zeros((P, P), dtype=BF)
                v = sc >= 0
                oh[np.arange(P)[v], sc[v].astype(np.int64)] = 1.0
                lhsm[j // LCH, :, j % LCH, :] = oh
        data.append(dict(idx=idxh, sid_d=sid_d, sid_a=sid_a, lhsm=lhsm))

    # ---- per-core canonical scalars / masks / y0 ----
    combo = (emb_type[:, None, :].astype(np.float64)
             + emb_inv[None, :, :].astype(np.float64)).reshape(12, D)
    cw = combo @ W0.astype(np.float64)             # [12, D]
    cid = node_type * 3 + ninv
    dinv2 = np.ones((NC, ncp), dtype=np.float32)
    dinvc = np.ones((NC, ncp), dtype=np.float32)
    gmask = np.zeros((NC, ncp, NG), dtype=BF)
    y0 = np.zeros((NC, ncp, D), dtype=np.float32)
    for c in range(NC):
        nodes_c = np.arange(c, N, NC)
        cp = canon_pos[nodes_c]
        dinv2[c, cp] = (dinv[nodes_c] ** 2).astype(np.float32)
        dinvc[c, cp] = dinv[nodes_c].astype(np.float32)
        gmask[c, cp, batch[nodes_c]] = 1.0
        y0[c, cp, :] = (cw[cid[nodes_c]] * dinv[nodes_c][:, None]).astype(np.float32)
        # pad slots mirror the cell head
        for g in range(NG):
            if cells[c][g] == 0:
                continue
            f = cell_start[g]
            for j in range(cell_start[g] + cells[c][g], cell_start[g] + T[g]):
                dinv2[c, j] = dinv2[c, f]
                dinvc[c, j] = dinvc[c, f]
                y0[c, j, :] = y0[c, f, :]
    table0 = y0.astype(BF)                          # [NC, ncp, D]
    ynm0 = np.zeros((NC, P, ntile, D), dtype=BF)
    for c in range(NC):
        ynm0[c] = table0[c].reshape(ntile, P, D).transpose(1, 0, 2)

    runs = [(int(cell_start[g]), int(cell_start[g + 1])) for g in range(NG)]
    return dict(ntile=ntile, ncp=ncp, ops=ops, gen_eng=gen_eng,
                dve_gi=dve_gi, act_gi=act_gi, dma_gi=dma_gi,
                n_dve_gen=n_dve_gen,
                n_act_gen=n_act_gen, n_dma=n_dma,
                nchunk_q=[int(x) for x in nchunk_q],
                nchmax=nchmax, data=data, dinv2=dinv2, dinvc=dinvc,
                gmask=gmask, table0=table0, ynm0=ynm0, runs=runs, cells=cells)


def _build(tpl):
    ntile, ncp = tpl["ntile"], tpl["ncp"]
    ops, gen_eng = tpl["ops"], tpl["gen_eng"]
    dve_gi, act_gi, dma_gi = tpl["dve_gi"], tpl["act_gi"], tpl["dma_gi"]
    n_dve_gen, n_act_gen, n_dma = tpl["n_dve_gen"], tpl["n_act_gen"], tpl["n_dma"]
    nchunk_q, nchmax = tpl["nchunk_q"], tpl["nchmax"]
    runs = tpl["runs"]
    n_agg = len(ops)
    n_dma_ch = -(-max(n_dma, 1) // LCH)
    dt = mybir.dt
    f32, bf16, i16 = dt.float32, dt.bfloat16, dt.int16

    tile_last_op = {}
    tile_ops = {}
    for i, (t, q, w) in enumerate(ops):
        tile_last_op[t] = i
        tile_ops.setdefault(t, []).append(i)
    for t in range(ntile):
        assert t in tile_ops, "tile without aggs unsupported"

    def tail_tile(b):
        t = ntile - 1
        while t % 4 != b:
            t -= 1
        return t

    # ---------- instruction sequences (shared by numbering and emission) ----
    def pe_seq(l):
        out = []
        for t in range(ntile):
            out.append(('ident', t))
            for i in tile_ops[t]:
                out.append(('agg', i))
            if t >= LAG:
                out.append(('tr', t - LAG))
                out.append(('xw', t - LAG))
        for t in range(ntile - LAG, ntile):
            out.append(('tr', t))
            out.append(('xw', t))
        return out

    def act_seq(l):
        out = []
        for t in range(ntile):
            for i in tile_ops[t]:
                if gen_eng[i] == 'act':
                    out.append(('gsq', i))
                    out.append(('grl', i))
            out.append(('ev', t))
            if t >= LAG:
                out.append(('vfm', t - LAG))
        for t in range(ntile - LAG, ntile):
            out.append(('vfm', t))
        return out

    def dve_seq(l):
        out = []
        for t in range(ntile):
            for i in tile_ops[t]:
                if gen_eng[i] == 'dve':
                    out.append(('gen', i))
            if l < 2 and t >= LAG:
                out.append(('ysc', t - LAG))
        if l < 2:
            for t in range(ntile - LAG, ntile):
                out.append(('ysc', t))
        else:
            for g in range(NG):
                out.append(('gmax', g))
        return out

    # ---------- numbering ----------
    pe_num, act_num, dve_num = {}, {}, {}
    k = 1  # pe #1 = iota transpose into ps_gen
    for l in range(3):
        for rec in pe_seq(l):
            k += 1
            pe_num[(l,) + rec] = k
    k = 0
    for l in range(3):
        for rec in act_seq(l):
            k += 1
            act_num[(l,) + rec] = k
    k += 1
    act_num['gsum'] = k
    k = 1  # memset
    for l in range(3):
        for rec in dve_seq(l):
            k += 1
            dve_num[(l,) + rec] = k

    # gather numbering (cumulative per quarter sem)
    gath_n = {}
    cnt_q = [0] * NQ
    for l in range(3):
        for ch in range(nchmax):
            for q in range(NQ):
                if ch < nchunk_q[q]:
                    cnt_q[q] += 1
                    gath_n[(l, q, ch)] = cnt_q[q]

    # msgs buffer recycle
    last_pe_of_chunk = {}
    for l in range(3):
        for i, (t, q, w) in enumerate(ops):
            last_pe_of_chunk[(l, q, w // CB)] = pe_num[(l, 'agg', i)]
    gath_wait_pe = {}
    prev_use = {}
    for l in range(3):
        for ch in range(nchmax):
            for q in range(NQ):
                if ch >= nchunk_q[q]:
                    continue
                par = ch % NBUFC
                gath_wait_pe[(l, q, ch)] = prev_use.get((q, par), 0)
                prev_use[(q, par)] = last_pe_of_chunk.get(
                    (l, q, ch), prev_use.get((q, par), 0))

    # generator/DMA lhs rings: slot + writer-wait (pe# of previous occupant)
    def ring_plan(idx_map, n_per_layer, depth):
        slot_of, wait_of = {}, {}
        prev = {}
        for l in range(3):
            for i in sorted(idx_map, key=idx_map.get):
                s = idx_map[i] % depth
                slot_of[(l, i)] = s
                if (ps := prev.get(s)) is not None:
                    wait_of[(l, i)] = pe_num[(ps[0], 'agg', ps[1])]
                prev[s] = (l, i)
        return slot_of, wait_of

    slot_dve, wait_dve = ring_plan(dve_gi, n_dve_gen, NLD)
    slot_act, wait_act = ring_plan(act_gi, n_act_gen, NLA)
    slot_dma, _ = ring_plan(dma_gi, n_dma, NLM)
    # lhs DMA chunks: (l, j) -> wait pe# for ring reuse
    inv_dma = {v: k2 for k2, v in dma_gi.items()}
    chunk_wait = {}
    prev_chunk = {}
    for l in range(3):
        for j in range(n_dma_ch):
            cslot = (j * LCH) % NLM
            if (pc := prev_chunk.get(cslot)) is not None:
                lp, jp = pc
                last = min(jp * LCH + LCH, n_dma) - 1
                chunk_wait[(l, j)] = pe_num[(lp, 'agg', inv_dma[last])]
            prev_chunk[cslot] = (l, j)

    # sync sdma numbering
    N_INIT = 13
    sd = N_INIT
    sd_lhs = {}
    SD_YST = {}
    for l in range(3):
        for j in range(n_dma_ch):
            sd += 1
            sd_lhs[(l, j)] = sd * 16
        if l < 2:
            sd += 1
            SD_YST[l] = sd * 16
    SD_TOTAL = (sd + 2) * 16
    SD_INIT = N_INIT * 16

    # ---------- build ----------
    nc = bacc.Bacc("TRN2", debug=False, num_swdge_queues=4, num_devices=NC)
    w1_d = nc.dram_tensor("w1b", [D, D], bf16, kind="ExternalInput")
    w2_d = nc.dram_tensor("w2b", [D, D], bf16, kind="ExternalInput")
    t0_d = nc.dram_tensor("table0", [NC, ncp, D], bf16, kind="ExternalInput")
    idx_d = nc.dram_tensor("idxh", [P, NQ, nchmax, CI // 16], i16,
                           kind="ExternalInput")
    sidd_d = nc.dram_tensor("sidd", [P, max(n_dve_gen, 1)], f32,
                            kind="ExternalInput")
    sida_d = nc.dram_tensor("sida", [P, max(n_act_gen, 1)], f32,
                            kind="ExternalInput")
    lhsm_d = nc.dram_tensor("lhsm", [n_dma_ch, P, LCH, P], bf16,
                            kind="ExternalInput")
    ynm0_d = nc.dram_tensor("ynm0", [P, ntile, D], bf16, kind="ExternalInput")
    dinv2_d = nc.dram_tensor("dinv2", [ntile, P], f32, kind="ExternalInput")
    dinvc_d = nc.dram_tensor("dinvc", [ntile, P], f32, kind="ExternalInput")
    gmask_d = nc.dram_tensor("gmask", [ntile, P, NG], bf16, kind="ExternalInput")
    ident_d = nc.dram_tensor("ident", [P, P], bf16, kind="ExternalInput")
    iota_d = nc.dram_tensor("iota", [P, P], bf16, kind="ExternalInput")
    iotaf_d = nc.dram_tensor("iotaf", [P, P], f32, kind="ExternalInput")
    iotap_d = nc.dram_tensor("iotap", [P, P], bf16, kind="ExternalInput")
    gsum_o = nc.dram_tensor("gsum", [NG, D], f32, kind="ExternalOutput")
    gmax_o = nc.dram_tensor("gmax", [P, NG], f32, kind="ExternalOutput")
    ystage = nc.dram_tensor("ystage", [ncp, D], bf16)
    tables = [t0_d] + [nc.dram_tensor(f"table{l}", [NC, ncp, D], bf16,
                                      addr_space="Shared") for l in (1, 2)]

    from contextlib import ExitStack
    es = ExitStack()
    block = es.enter_context(nc.Block())
    sb = lambda *a: es.enter_context(nc.sbuf_tensor(*a))
    ps = lambda *a: es.enter_context(nc.psum_tensor(*a))
    sem = lambda n: es.enter_context(nc.semaphore(n))

    msgs = sb("msgs", [P, NQ, NBUFC, CB, D], bf16)
    idx_sb = sb("idx_sb", [P, NQ, nchmax, CI // 16], i16)
    sidd_sb = sb("sidd_sb", [P, max(n_dve_gen, 1)], f32)
    sida_sb = sb("sida_sb", [P, max(n_act_gen, 1)], f32)
    lhsd = sb("lhsd", [P, NLD, P], bf16)
    lhsa = sb("lhsa", [P, NLA, P], bf16)
    lhsm = sb("lhsm_sb", [P, NLM, P], bf16)
    tmp_sb = sb("tmp_sb", [P, P], bf16)
    vnm = sb("vnm", [P, ntile, D], bf16)
    vfm = sb("vfm", [P, ntile, P], bf16)
    ynm = sb("ynm", [P, ntile, D], bf16)
    w_sb = sb("w_sb", [P, 2, D], bf16)
    dinv2_sb = sb("dinv2_sb", [P, ntile], f32)
    dinvc_sb = sb("dinvc_sb", [P, ntile], f32)
    gmask_sb = sb("gmask_sb", [P, ntile, NG], bf16)
    ident_sb = sb("ident_sb", [P, P], bf16)
    iota_sb = sb("iota_sb", [P, P], bf16)
    iotaf_sb = sb("iotaf_sb", [P, P], f32)
    iotap_sb = sb("iotap_sb", [P, P], bf16)
    gmax_sb = sb("gmax_sb", [P, NG], f32)
    gsum_sb = sb("gsum_sb", [NG, D], f32)
    ps_red = ps("ps_red", [P, 4, 512], f32)
    ps_aux = ps("ps_aux", [P, 2, 1024], bf16)
    ps_gs = ps("ps_gs", [NG, D], f32)
    ps_gen = ps("ps_gen", [P, P], bf16)
    sdma = sem("sdma")
    gsem = [sem(f"g{q}") for q in range(NQ)]
    pe_s = sem("pe")
    act_s = sem("act")
    dve_s = sem("dve")
    cc_s = sem("cc")

    if True:

        @block.sync
        def _(s):
            _anc = nc.allow_non_contiguous_dma(reason="one-time small loads")
            _anc.__enter__()
            s.dma_start(w_sb[:, 0, :], w1_d[:]).then_inc(sdma, 16)
            s.dma_start(w_sb[:, 1, :], w2_d[:]).then_inc(sdma, 16)
            s.dma_start(dinv2_sb[:], dinv2_d[:].rearrange("t p -> p t")).then_inc(sdma, 16)
            s.dma_start(dinvc_sb[:], dinvc_d[:].rearrange("t p -> p t")).then_inc(sdma, 16)
            s.dma_start(gmask_sb[:], gmask_d[:].rearrange("t p g -> p t g")).then_inc(sdma, 16)
            s.dma_start(ident_sb[:], ident_d[:]).then_inc(sdma, 16)
            s.dma_start(iota_sb[:], iota_d[:]).then_inc(sdma, 16)
            s.dma_start(iotaf_sb[:], iotaf_d[:]).then_inc(sdma, 16)
            s.dma_start(iotap_sb[:], iotap_d[:]).then_inc(sdma, 16)
            s.dma_start(idx_sb[:], idx_d[:]).then_inc(sdma, 16)
            s.dma_start(sidd_sb[:], sidd_d[:]).then_inc(sdma, 16)
            s.dma_start(sida_sb[:], sida_d[:]).then_inc(sdma, 16)
            s.dma_start(ynm[:], ynm0_d[:]).then_inc(sdma, 16)
            _anc.__exit__(None, None, None)
            for l in range(3):
                for j in range(n_dma_ch):
                    if (wp := chunk_wait.get((l, j))) is not None:
                        s.wait_ge(pe_s, wp)
                    cslot = (j * LCH) % NLM
                    s.dma_start(lhsm[:, cslot:cslot + LCH, :],
                                lhsm_d[j]).then_inc(sdma, 16)
                if l < 2:
                    s.wait_ge(dve_s, dve_num[(l, 'ysc', ntile - 1)])
                    s.dma_start(ystage[:].rearrange("(t p) f -> p t f", p=P),
                                ynm[:]).then_inc(sdma, 16)
            s.wait_ge(dve_s, dve_num[(2, 'gmax', NG - 1)])
            s.wait_ge(act_s, act_num['gsum'])
            s.dma_start(gsum_o[:], gsum_sb[:]).then_inc(sdma, 16)
            s.dma_start(gmax_o[:], gmax_sb[:]).then_inc(sdma, 16)
            s.wait_ge(sdma, SD_TOTAL)

        @block.gpsimd
        def _(gp):
            gp.load_library(mlp)
            gp.wait_ge(sdma, SD_INIT)
            for l in range(3):
                if l > 0:
                    gp.wait_ge(cc_s, l)
                for ch in range(nchmax):
                    for q in range(NQ):
                        if ch >= nchunk_q[q]:
                            continue
                        wpe = gath_wait_pe[(l, q, ch)]
                        if wpe:
                            gp.wait_ge(pe_s, wpe)
                        gp.dma_gather(
                            msgs[:, q, ch % NBUFC, :, :],
                            tables[l][2 * q:2 * q + 2].rearrange(
                                "a b c -> (a b) c"),
                            idx_sb[:, q, ch, :],
                            CI, CI, D,
                            transpose=False,
                            single_packet=False,
                            queue_num=q,
                        ).then_inc(gsem[q], 16)
                if l < 2:
                    gp.wait_ge(sdma, SD_YST[l])
                    gp.collective_compute(
                        "AllGather", mybir.AluOpType.bypass,
                        ins=[ystage[:]], outs=[tables[l + 1][:]],
                        replica_groups=[list(range(NC))],
                    ).then_inc(cc_s, 1)

        @block.tensor
        def _(pe):
            pe.wait_ge(sdma, SD_INIT)
            pe.transpose(out=ps_gen[:, :P], in_=iotap_sb[:],
                         identity=ident_sb[:]).then_inc(pe_s, 1)
            for l in range(3):
                for kind, a in pe_seq(l):
                    if kind == 'ident':
                        t = a
                        if t >= 4:
                            pe.wait_ge(act_s, act_num[(l, 'ev', t - 4)])
                        if l < 2 and t >= 8:
                            pe.wait_ge(dve_s, dve_num[(l, 'ysc', t - 8)])
                        if l > 0:
                            tb = tail_tile(t % 4) if t < 4 else t
                            pe.wait_ge(dve_s, dve_num[(l - 1, 'ysc', tb)])
                        pe.matmul(out=ps_red[:, t % 4, :D], lhsT=ident_sb[:],
                                  rhs=ynm[:, t, :], start=True,
                                  stop=False).then_inc(pe_s, 1)
                    elif kind == 'agg':
                        i = a
                        t, q, w = ops[i]
                        ch = w // CB
                        pe.wait_ge(gsem[q], 16 * gath_n[(l, q, ch)])
                        e = gen_eng[i]
                        if e == 'dve':
                            pe.wait_ge(dve_s, dve_num[(l, 'gen', i)])
                            lhs_ap = lhsd[:, slot_dve[(l, i)], :]
                        elif e == 'act':
                            pe.wait_ge(act_s, act_num[(l, 'grl', i)])
                            lhs_ap = lhsa[:, slot_act[(l, i)], :]
                        else:
                            pe.wait_ge(sdma, sd_lhs[(l, dma_gi[i] // LCH)])
                            lhs_ap = lhsm[:, slot_dma[(l, i)], :]
                        pe.matmul(out=ps_red[:, t % 4, :D], lhsT=lhs_ap,
                                  rhs=msgs[:, q, ch % NBUFC, w % CB, :],
                                  start=False,
                                  stop=(i == tile_last_op[t])).then_inc(pe_s, 1)
                    elif kind == 'tr':
                        t = a
                        pe.wait_ge(act_s, act_num[(l, 'ev', t)])
                        if t >= 2:
                            pe.wait_ge(act_s, act_num[(l, 'vfm', t - 2)])
                        pe.transpose(out=ps_aux[:, t % 2, :P], in_=vnm[:, t, :],
                                     identity=ident_sb[:]).then_inc(pe_s, 1)
                    else:  # 'xw'
                        t = a
                        if l < 2:
                            pe.wait_ge(act_s, act_num[(l, 'vfm', t)])
                            pe.matmul(out=ps_red[:, t % 4, :D],
                                      lhsT=vfm[:, t, :], rhs=w_sb[:, l, :],
                                      start=True, stop=True).then_inc(pe_s, 1)
                        else:
                            pe.wait_ge(act_s, act_num[(2, 'ev', t)])
                            pe.matmul(out=ps_gs[:], lhsT=gmask_sb[:, t, :],
                                      rhs=vnm[:, t, :], start=(t == 0),
                                      stop=(t == ntile - 1)).then_inc(pe_s, 1)

        @block.scalar
        def _(a_):
            Act = mybir.ActivationFunctionType
            a_.wait_ge(sdma, SD_INIT)
            for l in range(3):
                for kind, a in act_seq(l):
                    if kind == 'gsq':
                        i = a
                        if (wp := wait_act.get((l, i))) is not None:
                            a_.wait_ge(pe_s, wp)
                        gi = act_gi[i]
                        a_.activation(tmp_sb[:], iota_sb[:], Act.Square,
                                      bias=sida_sb[:, gi:gi + 1]).then_inc(act_s, 1)
                    elif kind == 'grl':
                        i = a
                        a_.activation(lhsa[:, slot_act[(l, i)], :], tmp_sb[:],
                                      Act.Relu, scale=-1.0,
                                      bias=1.0).then_inc(act_s, 1)
                    elif kind == 'ev':
                        t = a
                        a_.wait_ge(pe_s, pe_num[(l, 'agg', tile_last_op[t])])
                        if l < 2:
                            a_.activation(vnm[:, t, :], ps_red[:, t % 4, :D],
                                          Act.Relu).then_inc(act_s, 1)
                        else:
                            a_.activation(vnm[:, t, :], ps_red[:, t % 4, :D],
                                          Act.Copy,
                                          scale=dinvc_sb[:, t:t + 1]
                                          ).then_inc(act_s, 1)
                    else:  # 'vfm'
                        t = a
                        a_.wait_ge(pe_s, pe_num[(l, 'tr', t)])
                        a_.activation(vfm[:, t, :], ps_aux[:, t % 2, :P],
                                      Act.Copy).then_inc(act_s, 1)
            a_.wait_ge(pe_s, pe_num[(2, 'xw', ntile - 1)])
            a_.activation(gsum_sb[:], ps_gs[:], Act.Copy).then_inc(act_s, 1)

        @block.vector
        def _(v):
            v.memset(gmax_sb[:], -1e30).then_inc(dve_s, 1)
            v.wait_ge(sdma, SD_INIT)
            v.wait_ge(pe_s, 1)
            for l in range(3):
                for kind, a in dve_seq(l):
                    if kind == 'gen':
                        i = a
                        if (wp := wait_dve.get((l, i))) is not None:
                            v.wait_ge(pe_s, wp)
                        gi = dve_gi[i]
                        v.tensor_scalar(out=lhsd[:, slot_dve[(l, i)], :],
                                        in0=ps_gen[:, :P],
                                        scalar1=sidd_sb[:, gi:gi + 1],
                                        scalar2=None,
                                        op0=mybir.AluOpType.is_equal
                                        ).then_inc(dve_s, 1)
                    elif kind == 'ysc':
                        t = a
                        v.wait_ge(pe_s, pe_num[(l, 'xw', t)])
                        v.tensor_scalar(out=ynm[:, t, :],
                                        in0=ps_red[:, t % 4, :D],
                                        scalar1=dinv2_sb[:, t:t + 1],
                                        scalar2=None,
                                        op0=mybir.AluOpType.mult
                                        ).then_inc(dve_s, 1)
                    else:  # 'gmax'
                        g = a
                        a0, b0 = runs[g]
                        vf = vfm[:].rearrange("p t f -> p (t f)")
                        v.wait_ge(act_s, act_num[(2, 'vfm', (b0 - 1) // P)])
                        v.tensor_reduce(out=gmax_sb[:, g:g + 1],
                                        in_=vf[:, a0:b0],
                                        axis=mybir.AxisListType.X,
                                        op=mybir.AluOpType.max
                                        ).then_inc(dve_s, 1)

    es.close()
    nc.compile()
    return nc


def kernel(**inputs):
    node_type = np.asarray(inputs["node_type"]).astype(np.int64)
    ninv = np.asarray(inputs["num_inverted_predecessors"]).astype(np.int64)
    ei = np.asarray(inputs["edge_index"]).astype(np.int64)
    batch = np.asarray(inputs["batch"]).astype(np.int64)
    emb_type = np.asarray(inputs["emb_type"]).astype(np.float32)
    emb_inv = np.asarray(inputs["emb_inv"]).astype(np.float32)
    W0 = np.asarray(inputs["W0"]).astype(np.float32)
    W1 = np.asarray(inputs["W1"]).astype(np.float32)
    W2 = np.asarray(inputs["W2"]).astype(np.float32)

    tpl = _host_prep(node_type, ninv, ei[0], ei[1], batch, emb_type, emb_inv, W0)
    nc = _build(tpl)

    ident = np.eye(P, dtype=BF)
    iotaf = np.tile(np.arange(P, dtype=np.float32)[None, :], (P, 1))
    iota = iotaf.astype(BF)
    in_maps = []
    for c in range(NC):
        in_maps.append(dict(
            w1b=W1.astype(BF),
            w2b=W2.astype(BF),
            table0=tpl["table0"],
            idxh=tpl["data"][c]["idx"],
            sidd=tpl["data"][c]["sid_d"],
            sida=tpl["data"][c]["sid_a"],
            lhsm=tpl["data"][c]["lhsm"],
            ynm0=tpl["ynm0"][c],
            dinv2=tpl["dinv2"][c].reshape(tpl["ntile"], P),
            dinvc=tpl["dinvc"][c].reshape(tpl["ntile"], P),
            gmask=tpl["gmask"][c].reshape(tpl["ntile"], P, NG),
            ident=ident,
            iota=iota,
            iotaf=iotaf.astype(np.float32),
            iotap=np.tile(np.arange(P, dtype=np.float32)[:, None],
                          (1, P)).astype(BF),
        ))
    import os
    trace = os.environ.get("BASS_KERNEL_TRACE", "0") == "1"
    if trace:
        sys.path.insert(0, "/root/problem/work")
        try:
            import axon_trace_patch  # noqa
        except Exception:
            trace = False
    res = run_bass_kernel_spmd(nc, in_maps, core_ids=list(range(NC)), trace=trace)
    kernel.last_exec_ns = res.exec_time_ns

    gsum = np.zeros((NG, D), dtype=np.float64)
    gmax = np.full((NG, D), -np.inf)
    for c in range(NC):
        gsum += res.results[c]["gsum"].astype(np.float64)
        gm = res.results[c]["gmax"].astype(np.float64).T   # [NG, P]
        pres = tpl["cells"][c] > 0
        gmax[pres] = np.maximum(gmax[pres], gm[pres])
    out = np.concatenate([gmax, gsum], axis=1).astype(np.float32)
    return (np.round(out * 1000.0) / 1000.0).astype(np.float32)
